# revision 5
# baseline (speedup 1.0000x reference)
"""Causal multi-head attention layer (train forward) on 8 Trainium2 NeuronCores.

Sharding: batch (4) x head-group (2 of 8 heads each) -> 8 cores.
Each core computes, for its (batch b, head group g):
  Q^T,K^T [512, S] and V [S, 512] projections from x_b (bf16 compute, fp32 accum),
  causal flash-style attention with softmax rowsums obtained from a ones column
  appended to V (ctx matmul M=65), deferred normalization via DVE reciprocal,
  then a partial output projection with the Wo row-slice for its heads.
The host sums the two partials per batch and adds bo.
"""
import numpy as np

import concourse.bass as bass
import concourse.tile as tile
from concourse import bacc, mybir
from concourse.bass_utils import run_bass_kernel_spmd

F32 = mybir.dt.float32
BF16 = mybir.dt.bfloat16
AF = mybir.ActivationFunctionType
ALU = mybir.AluOpType

P = 128
D = 1024          # model dim
DC = 512          # per-core head dims (8 heads x 64)
HD = 64
NHC = 8           # heads per core
NPAIR = 4         # head pairs per core
FC = D // P       # 8 feature chunks
OC = DC // P      # 4 outdim chunks (= head pairs)
WIN = 1024        # sq window (must be <= 1024, multiple of 512)
SCALE = 1.0 / 32.0  # 1/sqrt(D)


def _copy(nc, i, out, in_):
    if i % 2 == 0:
        nc.vector.tensor_copy(out, in_)
    else:
        nc.scalar.copy(out, in_)


def _regions(vs, win):
    """512-aligned PSUM-bank regions of [vs, win)."""
    out = []
    for a in range(0, win, 512):
        b = a + 512
        if vs < b:
            out.append((max(vs, a), b))
    return out


def build_nc(S=2048, num_devices=8):
    TOKC = S // P
    win = min(WIN, S)
    NW = S // win
    NR = NPAIR * 2 * NW  # rowsum collector rows

    nc = bacc.Bacc("TRN2", target_bir_lowering=False, debug=False,
                   num_devices=num_devices)
    x = nc.dram_tensor("x", [S, D], F32, kind="ExternalInput").ap()
    wq = nc.dram_tensor("wq", [D, DC], F32, kind="ExternalInput").ap()
    wk = nc.dram_tensor("wk", [D, DC], F32, kind="ExternalInput").ap()
    wv = nc.dram_tensor("wv", [D, DC], F32, kind="ExternalInput").ap()
    wo = nc.dram_tensor("wo", [DC, D], F32, kind="ExternalInput").ap()
    bq = nc.dram_tensor("bq", [DC], F32, kind="ExternalInput").ap()
    bk = nc.dram_tensor("bk", [DC], F32, kind="ExternalInput").ap()
    bv = nc.dram_tensor("bv", [DC], F32, kind="ExternalInput").ap()
    ident = nc.dram_tensor("ident", [P, P], F32, kind="ExternalInput").ap()
    tri = nc.dram_tensor("tri", [P, P], F32, kind="ExternalInput").ap()
    out = nc.dram_tensor("out", [S, D], F32, kind="ExternalOutput").ap()

    with tile.TileContext(nc) as tc:
        with tc.tile_pool(name="const", bufs=1) as cst, \
             tc.tile_pool(name="stage", bufs=3) as stg, \
             tc.tile_pool(name="pt", bufs=2) as ptp, \
             tc.tile_pool(name="small", bufs=2) as sml, \
             tc.tile_pool(name="mm", bufs=1, space="PSUM") as mmp, \
             tc.tile_pool(name="ctxp", bufs=1, space="PSUM") as ctxpp:

            def mm_tile(i):
                return mmp.tile([P, 1024], F32, tag=f"s{i % 2}",
                                name=f"mm_s{i % 2}")

            # --- constants ---
            ident_sb = cst.tile([P, P], F32, tag="ident")
            nc.sync.dma_start(ident_sb[:], ident[:])
            tri_st = stg.tile([P, P], F32, tag="wstage")
            nc.sync.dma_start(tri_st[:], tri[:])
            tri_bf = cst.tile([P, P], BF16, tag="tri")
            nc.vector.tensor_copy(tri_bf[:], tri_st[:])
            bq_sb = cst.tile([P, OC], F32, tag="bq")
            nc.sync.dma_start(bq_sb[:], bq.rearrange("(c p) -> p c", p=P))
            bk_sb = cst.tile([P, OC], F32, tag="bk")
            nc.sync.dma_start(bk_sb[:], bk.rearrange("(c p) -> p c", p=P))
            bv_sb = cst.tile([P, OC], F32, tag="bv")
            nc.sync.dma_start(bv_sb[:], bv.rearrange("(c p) -> p c", p=P))

            # --- weights -> bf16 ---
            w_sbs = {}
            for name, wdram in (("wq", wq), ("wk", wk), ("wv", wv)):
                w_sb = cst.tile([P, FC, DC], BF16, tag=name)
                w_sbs[name] = w_sb
                for fc in range(FC):
                    st = stg.tile([P, DC], F32, tag="wstage2")
                    nc.sync.dma_start(st[:], wdram[fc * P:(fc + 1) * P, :])
                    _copy(nc, fc, w_sb[:, fc, :], st[:])
            wo_sb = cst.tile([P, OC, D], BF16, tag="wo")
            for c in range(OC):
                st = stg.tile([P, D], F32, tag="xstage")
                nc.sync.dma_start(st[:], wo[c * P:(c + 1) * P, :])
                _copy(nc, c, wo_sb[:, c, :], st[:])

            # --- x^T (feature-major, bf16) via PE transpose ---
            x_T = cst.tile([P, FC, S], BF16, tag="xT")
            for tokc in range(TOKC):
                st = stg.tile([P, D], F32, tag="xstage")
                nc.sync.dma_start(st[:], x[tokc * P:(tokc + 1) * P, :])
                ps = mm_tile(tokc)
                for fc in range(FC):
                    nc.tensor.transpose(ps[:, fc * P:(fc + 1) * P],
                                        st[:, fc * P:(fc + 1) * P], ident_sb[:])
                _copy(nc, tokc, x_T[:, :, tokc * P:(tokc + 1) * P], ps[:])

            # --- Q^T / K^T projections ---
            qT = cst.tile([P, OC, S], BF16, tag="qT")
            kT = cst.tile([P, OC, S], BF16, tag="kT")
            n1024 = S // 1024 if S >= 1024 else 1
            tw_w = min(1024, S)
            for dst, wname, b_sb in ((qT, "wq", bq_sb), (kT, "wk", bk_sb)):
                w_sb = w_sbs[wname]
                for oc in range(OC):
                    for th in range(n1024):
                        ps = mm_tile(oc + th)
                        for a in range(0, tw_w, 512):
                            nw = min(512, tw_w - a)
                            for fc in range(FC):
                                nc.tensor.matmul(
                                    ps[:, a:a + nw],
                                    w_sb[:, fc, oc * P:(oc + 1) * P],
                                    x_T[:, fc, th * tw_w + a: th * tw_w + a + nw],
                                    start=(fc == 0), stop=(fc == FC - 1))
                        eng = nc.vector if (oc + th) % 2 == 0 else nc.scalar
                        if eng is nc.vector:
                            eng.tensor_scalar(dst[:, oc, th * tw_w:(th + 1) * tw_w],
                                              ps[:, :tw_w], b_sb[:, oc:oc + 1], None,
                                              ALU.add)
                        else:
                            eng.activation(dst[:, oc, th * tw_w:(th + 1) * tw_w],
                                           ps[:, :tw_w], AF.Identity,
                                           bias=b_sb[:, oc:oc + 1])

            # --- V (token-major, with ones column at 64) ---
            v_sb = cst.tile([P, TOKC, NHC, HD + 1], BF16, tag="v")
            nc.vector.memset(v_sb[:, :, :, HD:HD + 1], 1.0)
            for tokc in range(TOKC):
                ps = mm_tile(tokc)
                for fc in range(FC):
                    nc.tensor.matmul(ps[:, 0:DC],
                                     x_T[:, fc, tokc * P:(tokc + 1) * P],
                                     w_sbs["wv"][:, fc, :],
                                     start=(fc == 0), stop=(fc == FC - 1))
                _copy(nc, tokc, v_sb[:, tokc, :, 0:HD], ps[:, 0:DC])

            # --- attention ---
            ctx_sb = cst.tile([P, NPAIR, S], BF16, tag="ctx")
            rs_all = cst.tile([NR, win], F32, tag="rs")
            for p in range(NPAIR):
                for w in range(NW):
                    ctx0 = ctxpp.tile([P, 1024], F32, tag="ctx0")
                    ctx1 = ctxpp.tile([P, 1024], F32, tag="ctx1")
                    skc_hi = (w + 1) * win // P
                    for skc in range(skc_hi):
                        rel = skc * P - w * win
                        vs = max(rel, 0)
                        regs = _regions(vs, win)
                        strip0 = mmp.tile([P, win], F32, tag="s0")
                        strip1 = mmp.tile([P, win], F32, tag="s1")
                        for (a, b) in regs:
                            nc.tensor.matmul(
                                strip0[:, a:b],
                                kT[0:HD, p, skc * P:(skc + 1) * P],
                                qT[0:HD, p, w * win + a: w * win + b],
                                start=True, stop=True)
                            nc.tensor.matmul(
                                strip1[:, a:b],
                                kT[HD:P, p, skc * P:(skc + 1) * P],
                                qT[HD:P, p, w * win + a: w * win + b],
                                start=True, stop=True)
                        pt0 = ptp.tile([P, win], BF16, tag="pt0")
                        pt1 = ptp.tile([P, win], BF16, tag="pt1")
                        nc.scalar.activation(pt0[:, vs:win], strip0[:, vs:win],
                                             AF.Exp, scale=SCALE)
                        nc.scalar.activation(pt1[:, vs:win], strip1[:, vs:win],
                                             AF.Exp, scale=SCALE)
                        if rel >= 0:
                            nc.vector.tensor_tensor(pt0[:, rel:rel + P],
                                                    pt0[:, rel:rel + P],
                                                    tri_bf[:], ALU.mult)
                            nc.vector.tensor_tensor(pt1[:, rel:rel + P],
                                                    pt1[:, rel:rel + P],
                                                    tri_bf[:], ALU.mult)
                        for (a, b) in regs:
                            last = (w * win + b) // P - 1
                            nc.tensor.matmul(ctx0[0:HD + 1, a:b],
                                             v_sb[:, skc, 2 * p, :],
                                             pt0[:, a:b],
                                             start=(skc == 0), stop=(skc == last))
                            nc.tensor.matmul(ctx1[0:HD + 1, a:b],
                                             v_sb[:, skc, 2 * p + 1, :],
                                             pt1[:, a:b],
                                             start=(skc == 0), stop=(skc == last))
                    # drains (unnormalized); rowsums to collector
                    r0 = (p * 2 + 0) * NW + w
                    r1 = (p * 2 + 1) * NW + w
                    row0 = sml.tile([1, win], F32, tag="row0")
                    nc.vector.tensor_copy(row0[:], ctx0[HD:HD + 1, :win])
                    nc.sync.dma_start(rs_all[r0:r0 + 1, :], row0[:])
                    row1 = sml.tile([1, win], F32, tag="row1")
                    nc.vector.tensor_copy(row1[:], ctx1[HD:HD + 1, :win])
                    nc.sync.dma_start(rs_all[r1:r1 + 1, :], row1[:])
                    nc.vector.tensor_copy(ctx_sb[0:HD, p, w * win:(w + 1) * win],
                                          ctx0[0:HD, :win])
                    stg1 = sml.tile([HD, win], BF16, tag="stg1")
                    nc.vector.tensor_copy(stg1[:], ctx1[0:HD, :win])
                    nc.sync.dma_start(ctx_sb[HD:P, p, w * win:(w + 1) * win],
                                      stg1[:])

            # --- deferred softmax normalization ---
            rcp_all = cst.tile([NR, win], F32, tag="rcp")
            nc.vector.reciprocal(rcp_all[:], rs_all[:])
            for p in range(NPAIR):
                for h in range(2):
                    for w in range(NW):
                        r = (p * 2 + h) * NW + w
                        bc = sml.tile([P, win], F32, tag="bc")
                        bch = bc[h * HD:(h + 1) * HD, :]
                        nc.sync.dma_start(
                            bch, rcp_all[r:r + 1, None, :].to_broadcast(
                                [1, HD, win]))
                        sl = ctx_sb[h * HD:(h + 1) * HD, p,
                                    w * win:(w + 1) * win]
                        nc.vector.tensor_tensor(sl, sl, bch, ALU.mult)
                        nc.vector.tensor_scalar(
                            sl, sl, bv_sb[h * HD:(h + 1) * HD, p:p + 1], None,
                            ALU.add)

            # --- output projection (partial over this core's heads) ---
            for tokc in range(TOKC):
                ps = mm_tile(tokc)
                for nb in range(2):
                    for pr in range(NPAIR):
                        nc.tensor.matmul(
                            ps[:, nb * 512:(nb + 1) * 512],
                            ctx_sb[:, pr, tokc * P:(tokc + 1) * P],
                            wo_sb[:, pr, nb * 512:(nb + 1) * 512],
                            start=(pr == 0), stop=(pr == NPAIR - 1))
                ost = stg.tile([P, D], F32, tag="ostage")
                _copy(nc, tokc, ost[:], ps[:])
                nc.sync.dma_start(out[tokc * P:(tokc + 1) * P, :], ost[:])

    nc.compile()
    return nc


def make_in_maps(x, Wq, bq, Wk, bk, Wv, bv, Wo):
    ident = np.eye(P, dtype=np.float32)
    # tri[p, f] = 1 where f >= p  (keep key p for queries f within diag block)
    tri = np.triu(np.ones((P, P), dtype=np.float32))
    in_maps = []
    for c in range(8):
        b, g = c // 2, c % 2
        sl = slice(g * DC, (g + 1) * DC)
        in_maps.append({
            "x": np.ascontiguousarray(x[b]),
            "wq": np.ascontiguousarray(Wq[:, sl]),
            "wk": np.ascontiguousarray(Wk[:, sl]),
            "wv": np.ascontiguousarray(Wv[:, sl]),
            "wo": np.ascontiguousarray(Wo[sl, :]),
            "bq": np.ascontiguousarray(bq[sl]),
            "bk": np.ascontiguousarray(bk[sl]),
            "bv": np.ascontiguousarray(bv[sl]),
            "ident": ident,
            "tri": tri,
        })
    return in_maps


_NC_CACHE = {}


def kernel(x, Wq, bq, Wk, bk, Wv, bv, Wo, bo):
    x = np.asarray(x, dtype=np.float32)
    args = [np.asarray(a, dtype=np.float32)
            for a in (Wq, bq, Wk, bk, Wv, bv, Wo, bo)]
    Wq, bq, Wk, bk, Wv, bv, Wo, bo = args
    if "nc" not in _NC_CACHE:
        _NC_CACHE["nc"] = build_nc(S=x.shape[1], num_devices=8)
    nc = _NC_CACHE["nc"]
    in_maps = make_in_maps(x, Wq, bq, Wk, bk, Wv, bv, Wo)
    res = run_bass_kernel_spmd(nc, in_maps, core_ids=list(range(8)))
    B = x.shape[0]
    out = np.empty_like(x)
    for b in range(B):
        out[b] = res.results[2 * b]["out"] + res.results[2 * b + 1]["out"] + bo
    return out


# revision 7
# speedup vs baseline: 1.0379x; 1.0379x over previous
"""Causal multi-head attention layer (train forward) on 8 Trainium2 NeuronCores.

Sharding: batch (4) x head-group (2 of 8 heads each) -> 8 cores.
Per core (batch b, head group g): project Q^T/K^T [512,S] and V [S,512] from
x_b in bf16 (fp32 PSUM accum), run causal attention head-pair-packed on the PE
array (row tiles at partition 0/64), softmax rowsums ride a ones column on V
(ctx matmul M=65), normalization is a DVE reciprocal + DMA row broadcast fused
into the ctx PSUM drain, then a partial output projection with this core's Wo
row block. Host sums the two partials per batch and adds bo.

Everything is emitted window-major (512 query tokens) so projections,
attention, normalization and the output projection of adjacent windows
pipeline across engines.
"""
import numpy as np

import concourse.bass as bass
import concourse.tile as tile
from concourse import bacc, mybir
from concourse.bass_utils import run_bass_kernel_spmd

F32 = mybir.dt.float32
BF16 = mybir.dt.bfloat16
AF = mybir.ActivationFunctionType
ALU = mybir.AluOpType

P = 128
D = 1024          # model dim
DC = 512          # per-core head dims (8 heads x 64)
HD = 64
NHC = 8           # heads per core
NPAIR = 4         # head pairs per core
FC = D // P       # 8 feature chunks
OC = DC // P      # 4 outdim chunks (= head pairs)
W = 512           # window width (1 PSUM bank of fp32)
WT = W // P       # token chunks per window
SCALE = 1.0 / 32.0  # 1/sqrt(D)


def _copy(nc, i, out, in_):
    if i % 2 == 0:
        nc.vector.tensor_copy(out, in_)
    else:
        nc.scalar.copy(out, in_)


def build_nc(S=2048, num_devices=8):
    NWIN = S // W

    nc = bacc.Bacc("TRN2", target_bir_lowering=False, debug=False,
                   num_devices=num_devices)
    x = nc.dram_tensor("x", [S, D], F32, kind="ExternalInput").ap()
    wq = nc.dram_tensor("wq", [D, DC], F32, kind="ExternalInput").ap()
    wk = nc.dram_tensor("wk", [D, DC], F32, kind="ExternalInput").ap()
    wv = nc.dram_tensor("wv", [D, DC], F32, kind="ExternalInput").ap()
    wo = nc.dram_tensor("wo", [DC, D], F32, kind="ExternalInput").ap()
    bq = nc.dram_tensor("bq", [DC], F32, kind="ExternalInput").ap()
    bk = nc.dram_tensor("bk", [DC], F32, kind="ExternalInput").ap()
    bv = nc.dram_tensor("bv", [DC], F32, kind="ExternalInput").ap()
    ident = nc.dram_tensor("ident", [P, P], F32, kind="ExternalInput").ap()
    tri = nc.dram_tensor("tri", [P, P], F32, kind="ExternalInput").ap()
    out = nc.dram_tensor("out", [S, D], F32, kind="ExternalOutput").ap()

    with tile.TileContext(nc) as tc:
        with tc.tile_pool(name="const", bufs=1) as cst, \
             tc.tile_pool(name="stage", bufs=3) as stg, \
             tc.tile_pool(name="pt", bufs=3) as ptp, \
             tc.tile_pool(name="small", bufs=2) as sml, \
             tc.tile_pool(name="psA", bufs=2, space="PSUM") as psA, \
             tc.tile_pool(name="psC", bufs=2, space="PSUM") as psC:

            mm_ctr = [0]

            def mm_tile():
                i = mm_ctr[0]
                mm_ctr[0] += 1
                return psA.tile([P, W], F32, tag=f"s{i % 2}",
                                name=f"mm_s{i % 2}")

            # --- constants ---
            ident_sb = cst.tile([P, P], F32, tag="ident")
            nc.sync.dma_start(ident_sb[:], ident[:])
            tri_st = stg.tile([P, P], F32, tag="wstage")
            nc.sync.dma_start(tri_st[:], tri[:])
            tri_bf = cst.tile([P, P], BF16, tag="tri")
            nc.vector.tensor_copy(tri_bf[:], tri_st[:])
            bq_sb = cst.tile([P, OC], F32, tag="bq")
            nc.sync.dma_start(bq_sb[:], bq.rearrange("(c p) -> p c", p=P))
            bk_sb = cst.tile([P, OC], F32, tag="bk")
            nc.sync.dma_start(bk_sb[:], bk.rearrange("(c p) -> p c", p=P))
            bv_sb = cst.tile([HD, NHC], F32, tag="bv")
            nc.sync.dma_start(bv_sb[:], bv.rearrange("(h p) -> p h", p=HD))

            # --- weights -> bf16 ---
            w_sbs = {}
            for name, wdram in (("wq", wq), ("wk", wk), ("wv", wv)):
                w_sb = cst.tile([P, FC, DC], BF16, tag=name)
                w_sbs[name] = w_sb
                for fc in range(FC):
                    st = stg.tile([P, DC], F32, tag="wstage2")
                    nc.sync.dma_start(st[:], wdram[fc * P:(fc + 1) * P, :])
                    _copy(nc, fc, w_sb[:, fc, :], st[:])
            wo_sb = cst.tile([P, OC, D], BF16, tag="wo")
            for c in range(OC):
                st = stg.tile([P, D], F32, tag="xstage")
                nc.sync.dma_start(st[:], wo[c * P:(c + 1) * P, :])
                _copy(nc, c, wo_sb[:, c, :], st[:])

            # --- per-window tiles ---
            xT_w, qT_w, kT_w, v_w, ctx_w = [], [], [], [], []
            for j in range(NWIN):
                xT_w.append(cst.tile([P, FC, W], BF16, tag=f"xT{j}", name=f"xT{j}"))
                qT_w.append(cst.tile([P, OC, W], BF16, tag=f"qT{j}", name=f"qT{j}"))
                kT_w.append(cst.tile([P, OC, W], BF16, tag=f"kT{j}", name=f"kT{j}"))
                v_w.append(cst.tile([P, WT, NHC, HD + 1], BF16, tag=f"v{j}", name=f"v{j}"))
                ctx_w.append(cst.tile([P, NPAIR, W], BF16, tag=f"ctx{j}", name=f"ctx{j}"))
                nc.vector.memset(v_w[j][:, :, :, HD:HD + 1], 1.0)

            eng_ctr = [0]

            def nxt():
                eng_ctr[0] += 1
                return eng_ctr[0]

            for j in range(NWIN):
                # --- x^T for window j (PE transpose, fp32 -> bf16 on drain) ---
                for t in range(WT):
                    tokc = j * WT + t
                    st = stg.tile([P, D], F32, tag="xstage")
                    nc.sync.dma_start(st[:], x[tokc * P:(tokc + 1) * P, :])
                    for half in range(2):
                        ps = mm_tile()
                        for q in range(4):
                            fc = half * 4 + q
                            nc.tensor.transpose(ps[:, q * P:(q + 1) * P],
                                                st[:, fc * P:(fc + 1) * P],
                                                ident_sb[:])
                        _copy(nc, nxt(),
                              xT_w[j][:, half * 4:(half + 1) * 4,
                                      t * P:(t + 1) * P], ps[:])

                # --- Q^T / K^T for window j ---
                for dst, wname, b_sb in ((qT_w[j], "wq", bq_sb),
                                         (kT_w[j], "wk", bk_sb)):
                    w_sb = w_sbs[wname]
                    for oc in range(OC):
                        ps = mm_tile()
                        for fc in range(FC):
                            nc.tensor.matmul(
                                ps[:], w_sb[:, fc, oc * P:(oc + 1) * P],
                                xT_w[j][:, fc, :],
                                start=(fc == 0), stop=(fc == FC - 1))
                        if nxt() % 2 == 0:
                            nc.vector.tensor_scalar(
                                dst[:, oc, :], ps[:], b_sb[:, oc:oc + 1],
                                None, ALU.add)
                        else:
                            nc.scalar.activation(
                                dst[:, oc, :], ps[:], AF.Identity,
                                bias=b_sb[:, oc:oc + 1])

                # --- V for window j ---
                for t in range(WT):
                    ps = mm_tile()
                    for fc in range(FC):
                        nc.tensor.matmul(ps[:],
                                         xT_w[j][:, fc, t * P:(t + 1) * P],
                                         w_sbs["wv"][:, fc, :],
                                         start=(fc == 0), stop=(fc == FC - 1))
                    _copy(nc, nxt(), v_w[j][:, t, :, 0:HD], ps[:])

                # --- attention: all pairs, window j ---
                skc_hi = WT * (j + 1)
                for p in range(NPAIR):
                    ctx0 = psC.tile([P, W], F32, tag="c0", name="ctx0")
                    ctx1 = psC.tile([P, W], F32, tag="c1", name="ctx1")
                    for skc in range(skc_hi):
                        jk, tk = divmod(skc, WT)
                        rel = skc * P - j * W
                        vs = max(rel, 0)
                        strip0 = psA.tile([P, W], F32, tag="s0", name="strip0")
                        strip1 = psA.tile([P, W], F32, tag="s1", name="strip1")
                        nc.tensor.matmul(strip0[:, vs:W],
                                         kT_w[jk][0:HD, p, tk * P:(tk + 1) * P],
                                         qT_w[j][0:HD, p, vs:W],
                                         start=True, stop=True)
                        nc.tensor.matmul(strip1[:, vs:W],
                                         kT_w[jk][HD:P, p, tk * P:(tk + 1) * P],
                                         qT_w[j][HD:P, p, vs:W],
                                         start=True, stop=True)
                        pt0 = ptp.tile([P, W], BF16, tag="pt0", name="pt0")
                        pt1 = ptp.tile([P, W], BF16, tag="pt1", name="pt1")
                        nc.scalar.activation(pt0[:, vs:W], strip0[:, vs:W],
                                             AF.Exp, scale=SCALE)
                        nc.scalar.activation(pt1[:, vs:W], strip1[:, vs:W],
                                             AF.Exp, scale=SCALE)
                        if rel >= 0:
                            nc.vector.tensor_tensor(pt0[:, rel:rel + P],
                                                    pt0[:, rel:rel + P],
                                                    tri_bf[:], ALU.mult)
                            nc.vector.tensor_tensor(pt1[:, rel:rel + P],
                                                    pt1[:, rel:rel + P],
                                                    tri_bf[:], ALU.mult)
                        st0 = (skc == 0)
                        sp0 = (skc == skc_hi - 1)
                        nc.tensor.matmul(ctx0[0:HD + 1, vs:W],
                                         v_w[jk][:, tk, 2 * p, :],
                                         pt0[:, vs:W], start=st0, stop=sp0)
                        nc.tensor.matmul(ctx1[0:HD + 1, vs:W],
                                         v_w[jk][:, tk, 2 * p + 1, :],
                                         pt1[:, vs:W], start=st0, stop=sp0)

                    # normalization fused into the drain
                    rs0 = sml.tile([1, W], F32, tag="rs0")
                    nc.vector.reciprocal(rs0[:], ctx0[HD:HD + 1, :])
                    rs1 = sml.tile([1, W], F32, tag="rs1")
                    nc.vector.reciprocal(rs1[:], ctx1[HD:HD + 1, :])
                    bc0 = sml.tile([HD, W], F32, tag="bc0")
                    nc.sync.dma_start(
                        bc0[:], rs0[0:1, None, :].to_broadcast([1, HD, W]))
                    bc1 = sml.tile([HD, W], F32, tag="bc1")
                    nc.sync.dma_start(
                        bc1[:], rs1[0:1, None, :].to_broadcast([1, HD, W]))
                    dst0 = ctx_w[j][0:HD, p, :]
                    nc.vector.tensor_tensor(dst0, ctx0[0:HD, :], bc0[:],
                                            ALU.mult)
                    nc.vector.tensor_scalar(dst0, dst0,
                                            bv_sb[:, 2 * p:2 * p + 1], None,
                                            ALU.add)
                    stg1 = sml.tile([HD, W], BF16, tag="stg1")
                    nc.vector.tensor_tensor(stg1[:], ctx1[0:HD, :], bc1[:],
                                            ALU.mult)
                    nc.vector.tensor_scalar(stg1[:], stg1[:],
                                            bv_sb[:, 2 * p + 1:2 * p + 2],
                                            None, ALU.add)
                    nc.sync.dma_start(ctx_w[j][HD:P, p, :], stg1[:])

                # --- output projection for window j's tokens ---
                for t in range(WT):
                    tokc = j * WT + t
                    for nb in range(2):
                        ps = mm_tile()
                        for pr in range(NPAIR):
                            nc.tensor.matmul(
                                ps[:], ctx_w[j][:, pr, t * P:(t + 1) * P],
                                wo_sb[:, pr, nb * 512:(nb + 1) * 512],
                                start=(pr == 0), stop=(pr == NPAIR - 1))
                        ost = stg.tile([P, 512], F32, tag="ostage")
                        _copy(nc, nxt(), ost[:], ps[:])
                        nc.sync.dma_start(
                            out[tokc * P:(tokc + 1) * P,
                                nb * 512:(nb + 1) * 512], ost[:])

    nc.compile()
    return nc


def make_in_maps(x, Wq, bq, Wk, bk, Wv, bv, Wo):
    ident = np.eye(P, dtype=np.float32)
    # tri[p, f] = 1 where f >= p (keep key p for query f within a diag block)
    tri = np.triu(np.ones((P, P), dtype=np.float32))
    in_maps = []
    for c in range(8):
        b, g = c // 2, c % 2
        sl = slice(g * DC, (g + 1) * DC)
        in_maps.append({
            "x": np.ascontiguousarray(x[b]),
            "wq": np.ascontiguousarray(Wq[:, sl]),
            "wk": np.ascontiguousarray(Wk[:, sl]),
            "wv": np.ascontiguousarray(Wv[:, sl]),
            "wo": np.ascontiguousarray(Wo[sl, :]),
            "bq": np.ascontiguousarray(bq[sl]),
            "bk": np.ascontiguousarray(bk[sl]),
            "bv": np.ascontiguousarray(bv[sl]),
            "ident": ident,
            "tri": tri,
        })
    return in_maps


_NC_CACHE = {}


def kernel(x, Wq, bq, Wk, bk, Wv, bv, Wo, bo):
    x = np.asarray(x, dtype=np.float32)
    args = [np.asarray(a, dtype=np.float32)
            for a in (Wq, bq, Wk, bk, Wv, bv, Wo, bo)]
    Wq, bq, Wk, bk, Wv, bv, Wo, bo = args
    if "nc" not in _NC_CACHE:
        _NC_CACHE["nc"] = build_nc(S=x.shape[1], num_devices=8)
    nc = _NC_CACHE["nc"]
    in_maps = make_in_maps(x, Wq, bq, Wk, bk, Wv, bv, Wo)
    res = run_bass_kernel_spmd(nc, in_maps, core_ids=list(range(8)))
    B = x.shape[0]
    out = np.empty_like(x)
    for b in range(B):
        out[b] = res.results[2 * b]["out"] + res.results[2 * b + 1]["out"] + bo
    return out


# revision 9
# speedup vs baseline: 1.0513x; 1.0129x over previous
"""Causal multi-head attention layer (train forward) on 8 Trainium2 NeuronCores.

Sharding: batch (4) x head-group (2 of 8 heads each) -> 8 cores.
Per core (batch b, head group g): project Q^T/K^T [512,S] and V [S,512] from
x_b in bf16 (fp32 PSUM accum), run causal attention head-pair-packed on the PE
array (row tiles at partition 0/64), softmax rowsums ride a ones column on V
(ctx matmul M=65), normalization is a DVE reciprocal + DMA row broadcast fused
into the ctx PSUM drain, then a partial output projection with this core's Wo
row block. Host sums the two partials per batch and adds bo.

Everything is emitted window-major (512 query tokens) so projections,
attention, normalization and the output projection of adjacent windows
pipeline across engines.
"""
import numpy as np

import concourse.bass as bass
import concourse.tile as tile
from concourse import bacc, mybir
from concourse.bass_utils import run_bass_kernel_spmd

F32 = mybir.dt.float32
BF16 = mybir.dt.bfloat16
AF = mybir.ActivationFunctionType
ALU = mybir.AluOpType

P = 128
D = 1024          # model dim
DC = 512          # per-core head dims (8 heads x 64)
HD = 64
NHC = 8           # heads per core
NPAIR = 4         # head pairs per core
FC = D // P       # 8 feature chunks
OC = DC // P      # 4 outdim chunks (= head pairs)
W = 512           # window width (1 PSUM bank of fp32)
WT = W // P       # token chunks per window
SCALE = 1.0 / 32.0  # 1/sqrt(D)


def _copy(nc, i, out, in_):
    if i % 2 == 0:
        nc.vector.tensor_copy(out, in_)
    else:
        nc.scalar.copy(out, in_)


def build_nc(S=2048, num_devices=8):
    NWIN = S // W

    nc = bacc.Bacc("TRN2", target_bir_lowering=False, debug=False,
                   num_devices=num_devices)
    x = nc.dram_tensor("x", [S, D], F32, kind="ExternalInput").ap()
    wq = nc.dram_tensor("wq", [D, DC], F32, kind="ExternalInput").ap()
    wk = nc.dram_tensor("wk", [D, DC], F32, kind="ExternalInput").ap()
    wv = nc.dram_tensor("wv", [D, DC], F32, kind="ExternalInput").ap()
    wo = nc.dram_tensor("wo", [DC, D], F32, kind="ExternalInput").ap()
    bq = nc.dram_tensor("bq", [DC], F32, kind="ExternalInput").ap()
    bk = nc.dram_tensor("bk", [DC], F32, kind="ExternalInput").ap()
    bv = nc.dram_tensor("bv", [DC], F32, kind="ExternalInput").ap()
    ident = nc.dram_tensor("ident", [P, P], F32, kind="ExternalInput").ap()
    tri = nc.dram_tensor("tri", [P, P], F32, kind="ExternalInput").ap()
    out = nc.dram_tensor("out", [S, D], F32, kind="ExternalOutput").ap()

    with tile.TileContext(nc) as tc:
        with tc.tile_pool(name="const", bufs=1) as cst, \
             tc.tile_pool(name="stage", bufs=3) as stg, \
             tc.tile_pool(name="pt", bufs=3) as ptp, \
             tc.tile_pool(name="small", bufs=2) as sml, \
             tc.tile_pool(name="psA", bufs=2, space="PSUM") as psA, \
             tc.tile_pool(name="psC", bufs=2, space="PSUM") as psC:

            mm_ctr = [0]

            def mm_tile():
                i = mm_ctr[0]
                mm_ctr[0] += 1
                return psA.tile([P, W], F32, tag=f"s{i % 2}",
                                name=f"mm_s{i % 2}")

            # --- constants ---
            ident_sb = cst.tile([P, P], F32, tag="ident")
            nc.sync.dma_start(ident_sb[:], ident[:])
            tri_st = stg.tile([P, P], F32, tag="wstage")
            nc.sync.dma_start(tri_st[:], tri[:])
            tri_bf = cst.tile([P, P], BF16, tag="tri")
            nc.vector.tensor_copy(tri_bf[:], tri_st[:])
            bq_sb = cst.tile([P, OC], F32, tag="bq")
            nc.sync.dma_start(bq_sb[:], bq.rearrange("(c p) -> p c", p=P))
            bk_sb = cst.tile([P, OC], F32, tag="bk")
            nc.sync.dma_start(bk_sb[:], bk.rearrange("(c p) -> p c", p=P))
            bv_sb = cst.tile([HD, NHC], F32, tag="bv")
            nc.sync.dma_start(bv_sb[:], bv.rearrange("(h p) -> p h", p=HD))

            # --- weights -> bf16 ---
            w_sbs = {}
            for name, wdram in (("wq", wq), ("wk", wk), ("wv", wv)):
                w_sb = cst.tile([P, FC, DC], BF16, tag=name)
                w_sbs[name] = w_sb
                for fc in range(FC):
                    st = stg.tile([P, DC], F32, tag="wstage2")
                    nc.gpsimd.dma_start(st[:], wdram[fc * P:(fc + 1) * P, :])
                    _copy(nc, fc, w_sb[:, fc, :], st[:])
            wo_sb = cst.tile([P, OC, D], BF16, tag="wo")
            for c in range(OC):
                st = stg.tile([P, D], F32, tag="wostage")
                nc.gpsimd.dma_start(st[:], wo[c * P:(c + 1) * P, :])
                _copy(nc, c, wo_sb[:, c, :], st[:])

            # --- per-window tiles ---
            xT_w, qT_w, kT_w, v_w, ctx_w = [], [], [], [], []
            for j in range(NWIN):
                xT_w.append(cst.tile([P, FC, W], BF16, tag=f"xT{j}", name=f"xT{j}"))
                qT_w.append(cst.tile([P, OC, W], BF16, tag=f"qT{j}", name=f"qT{j}"))
                kT_w.append(cst.tile([P, OC, W], BF16, tag=f"kT{j}", name=f"kT{j}"))
                v_w.append(cst.tile([P, WT, NHC, HD + 1], BF16, tag=f"v{j}", name=f"v{j}"))
                ctx_w.append(cst.tile([P, NPAIR, W], BF16, tag=f"ctx{j}", name=f"ctx{j}"))
                nc.vector.memset(v_w[j][:, :, :, HD:HD + 1], 1.0)

            eng_ctr = [0]

            def nxt():
                eng_ctr[0] += 1
                return eng_ctr[0]

            for j in range(NWIN):
                # --- x^T for window j (PE transpose, fp32 -> bf16 on drain) ---
                for t in range(WT):
                    tokc = j * WT + t
                    st = stg.tile([P, D], F32, tag="xstage")
                    dma_eng = nc.sync if tokc % 2 == 0 else nc.gpsimd
                    dma_eng.dma_start(st[:], x[tokc * P:(tokc + 1) * P, :])
                    for half in range(2):
                        ps = mm_tile()
                        for q in range(4):
                            fc = half * 4 + q
                            nc.tensor.transpose(ps[:, q * P:(q + 1) * P],
                                                st[:, fc * P:(fc + 1) * P],
                                                ident_sb[:])
                        _copy(nc, nxt(),
                              xT_w[j][:, half * 4:(half + 1) * 4,
                                      t * P:(t + 1) * P], ps[:])

                # --- Q^T / K^T for window j ---
                for dst, wname, b_sb in ((qT_w[j], "wq", bq_sb),
                                         (kT_w[j], "wk", bk_sb)):
                    w_sb = w_sbs[wname]
                    for oc in range(OC):
                        ps = mm_tile()
                        for fc in range(FC):
                            nc.tensor.matmul(
                                ps[:], w_sb[:, fc, oc * P:(oc + 1) * P],
                                xT_w[j][:, fc, :],
                                start=(fc == 0), stop=(fc == FC - 1))
                        if nxt() % 2 == 0:
                            nc.vector.tensor_scalar(
                                dst[:, oc, :], ps[:], b_sb[:, oc:oc + 1],
                                None, ALU.add)
                        else:
                            nc.scalar.activation(
                                dst[:, oc, :], ps[:], AF.Identity,
                                bias=b_sb[:, oc:oc + 1])

                # --- V for window j ---
                for t in range(WT):
                    ps = mm_tile()
                    for fc in range(FC):
                        nc.tensor.matmul(ps[:],
                                         xT_w[j][:, fc, t * P:(t + 1) * P],
                                         w_sbs["wv"][:, fc, :],
                                         start=(fc == 0), stop=(fc == FC - 1))
                    _copy(nc, nxt(), v_w[j][:, t, :, 0:HD], ps[:])

                # --- attention: all pairs, window j ---
                skc_hi = WT * (j + 1)
                for p in range(NPAIR):
                    ctx0 = psC.tile([P, W], F32, tag="c0", name="ctx0")
                    ctx1 = psC.tile([P, W], F32, tag="c1", name="ctx1")
                    for skc in range(skc_hi):
                        jk, tk = divmod(skc, WT)
                        rel = skc * P - j * W
                        vs = max(rel, 0)
                        strip0 = psA.tile([P, W], F32, tag="s0", name="strip0")
                        strip1 = psA.tile([P, W], F32, tag="s1", name="strip1")
                        nc.tensor.matmul(strip0[:, vs:W],
                                         kT_w[jk][0:HD, p, tk * P:(tk + 1) * P],
                                         qT_w[j][0:HD, p, vs:W],
                                         start=True, stop=True)
                        nc.tensor.matmul(strip1[:, vs:W],
                                         kT_w[jk][HD:P, p, tk * P:(tk + 1) * P],
                                         qT_w[j][HD:P, p, vs:W],
                                         start=True, stop=True)
                        pt0 = ptp.tile([P, W], BF16, tag="pt0", name="pt0")
                        pt1 = ptp.tile([P, W], BF16, tag="pt1", name="pt1")
                        nc.scalar.activation(pt0[:, vs:W], strip0[:, vs:W],
                                             AF.Exp, scale=SCALE)
                        nc.scalar.activation(pt1[:, vs:W], strip1[:, vs:W],
                                             AF.Exp, scale=SCALE)
                        if rel >= 0:
                            nc.vector.tensor_tensor(pt0[:, rel:rel + P],
                                                    pt0[:, rel:rel + P],
                                                    tri_bf[:], ALU.mult)
                            nc.vector.tensor_tensor(pt1[:, rel:rel + P],
                                                    pt1[:, rel:rel + P],
                                                    tri_bf[:], ALU.mult)
                        st0 = (skc == 0)
                        sp0 = (skc == skc_hi - 1)
                        nc.tensor.matmul(ctx0[0:HD + 1, vs:W],
                                         v_w[jk][:, tk, 2 * p, :],
                                         pt0[:, vs:W], start=st0, stop=sp0)
                        nc.tensor.matmul(ctx1[0:HD + 1, vs:W],
                                         v_w[jk][:, tk, 2 * p + 1, :],
                                         pt1[:, vs:W], start=st0, stop=sp0)

                    # normalization fused into the drain
                    rs0 = sml.tile([1, W], F32, tag="rs0")
                    nc.vector.reciprocal(rs0[:], ctx0[HD:HD + 1, :])
                    rs1 = sml.tile([1, W], F32, tag="rs1")
                    nc.vector.reciprocal(rs1[:], ctx1[HD:HD + 1, :])
                    bc0 = sml.tile([HD, W], F32, tag="bc0")
                    nc.sync.dma_start(
                        bc0[:], rs0[0:1, None, :].to_broadcast([1, HD, W]))
                    bc1 = sml.tile([HD, W], F32, tag="bc1")
                    nc.sync.dma_start(
                        bc1[:], rs1[0:1, None, :].to_broadcast([1, HD, W]))
                    dst0 = ctx_w[j][0:HD, p, :]
                    nc.vector.tensor_tensor(dst0, ctx0[0:HD, :], bc0[:],
                                            ALU.mult)
                    nc.vector.tensor_scalar(dst0, dst0,
                                            bv_sb[:, 2 * p:2 * p + 1], None,
                                            ALU.add)
                    stg1 = sml.tile([HD, W], BF16, tag="stg1")
                    nc.vector.tensor_tensor(stg1[:], ctx1[0:HD, :], bc1[:],
                                            ALU.mult)
                    nc.vector.tensor_scalar(stg1[:], stg1[:],
                                            bv_sb[:, 2 * p + 1:2 * p + 2],
                                            None, ALU.add)
                    nc.sync.dma_start(ctx_w[j][HD:P, p, :], stg1[:])

                # --- output projection for window j's tokens ---
                for t in range(WT):
                    tokc = j * WT + t
                    for nb in range(2):
                        ps = mm_tile()
                        for pr in range(NPAIR):
                            nc.tensor.matmul(
                                ps[:], ctx_w[j][:, pr, t * P:(t + 1) * P],
                                wo_sb[:, pr, nb * 512:(nb + 1) * 512],
                                start=(pr == 0), stop=(pr == NPAIR - 1))
                        ost = stg.tile([P, 512], F32, tag="ostage")
                        _copy(nc, nxt(), ost[:], ps[:])
                        oeng = nc.sync if (t + nb) % 2 == 0 else nc.gpsimd
                        oeng.dma_start(
                            out[tokc * P:(tokc + 1) * P,
                                nb * 512:(nb + 1) * 512], ost[:])

    nc.compile()
    return nc


def make_in_maps(x, Wq, bq, Wk, bk, Wv, bv, Wo):
    ident = np.eye(P, dtype=np.float32)
    # tri[p, f] = 1 where f >= p (keep key p for query f within a diag block)
    tri = np.triu(np.ones((P, P), dtype=np.float32))
    in_maps = []
    for c in range(8):
        b, g = c // 2, c % 2
        sl = slice(g * DC, (g + 1) * DC)
        in_maps.append({
            "x": np.ascontiguousarray(x[b]),
            "wq": np.ascontiguousarray(Wq[:, sl]),
            "wk": np.ascontiguousarray(Wk[:, sl]),
            "wv": np.ascontiguousarray(Wv[:, sl]),
            "wo": np.ascontiguousarray(Wo[sl, :]),
            "bq": np.ascontiguousarray(bq[sl]),
            "bk": np.ascontiguousarray(bk[sl]),
            "bv": np.ascontiguousarray(bv[sl]),
            "ident": ident,
            "tri": tri,
        })
    return in_maps


_NC_CACHE = {}


def kernel(x, Wq, bq, Wk, bk, Wv, bv, Wo, bo):
    x = np.asarray(x, dtype=np.float32)
    args = [np.asarray(a, dtype=np.float32)
            for a in (Wq, bq, Wk, bk, Wv, bv, Wo, bo)]
    Wq, bq, Wk, bk, Wv, bv, Wo, bo = args
    if "nc" not in _NC_CACHE:
        _NC_CACHE["nc"] = build_nc(S=x.shape[1], num_devices=8)
    nc = _NC_CACHE["nc"]
    in_maps = make_in_maps(x, Wq, bq, Wk, bk, Wv, bv, Wo)
    res = run_bass_kernel_spmd(nc, in_maps, core_ids=list(range(8)))
    B = x.shape[0]
    out = np.empty_like(x)
    for b in range(B):
        out[b] = res.results[2 * b]["out"] + res.results[2 * b + 1]["out"] + bo
    return out


# revision 13
# speedup vs baseline: 1.0630x; 1.0111x over previous
"""Causal multi-head attention layer (train forward) on 8 Trainium2 NeuronCores.

Sharding: batch (4) x head-group (2 of 8 heads each) -> 8 cores.
Per core (batch b, head group g): project Q^T/K^T [512,S] and V [S,512] from
x_b in bf16 (fp32 PSUM accum), run causal attention head-pair-packed on the PE
array (row tiles at partitions 0/64, one [128,1024] PSUM strip pair per key
chunk), softmax rowsums ride a ones column on V (ctx matmul M=65), the
normalization reciprocal is batched per window on DVE and applied during the
ctx drain, then a partial output projection with this core's Wo row block.
Host pre-casts weights/x to bf16, sums the two partials per batch, adds bo.
"""
import numpy as np
import ml_dtypes

import concourse.bass as bass
import concourse.tile as tile
from concourse import bacc, mybir
from concourse.bass_utils import run_bass_kernel_spmd

F32 = mybir.dt.float32
BF16 = mybir.dt.bfloat16
AF = mybir.ActivationFunctionType
ALU = mybir.AluOpType

P = 128
D = 1024          # model dim
DC = 512          # per-core head dims (8 heads x 64)
HD = 64
NHC = 8           # heads per core
NPAIR = 4         # head pairs per core
FC = D // P       # 8 feature chunks
OC = DC // P      # 4 outdim chunks (= head pairs)
W = 512           # query window (fp32 PSUM bank)
WT = W // P       # token chunks per window
SCALE = 1.0 / 32.0  # 1/sqrt(D)


def _copy(nc, i, out, in_):
    if i % 2 == 0:
        nc.vector.tensor_copy(out, in_)
    else:
        nc.scalar.copy(out, in_)


def build_nc(S=2048, num_devices=8, with_bv=False):
    NWIN = S // W

    nc = bacc.Bacc("TRN2", target_bir_lowering=False, debug=False,
                   num_devices=num_devices)
    x = nc.dram_tensor("x", [S, D], BF16, kind="ExternalInput").ap()
    wq = nc.dram_tensor("wq", [D, DC], BF16, kind="ExternalInput").ap()
    wk = nc.dram_tensor("wk", [D, DC], BF16, kind="ExternalInput").ap()
    wv = nc.dram_tensor("wv", [D, DC], BF16, kind="ExternalInput").ap()
    wo = nc.dram_tensor("wo", [DC, D], BF16, kind="ExternalInput").ap()
    bq = nc.dram_tensor("bq", [DC], F32, kind="ExternalInput").ap()
    bk = nc.dram_tensor("bk", [DC], F32, kind="ExternalInput").ap()
    bv = nc.dram_tensor("bv", [DC], F32, kind="ExternalInput").ap()
    ident = nc.dram_tensor("ident", [P, P], BF16, kind="ExternalInput").ap()
    tri = nc.dram_tensor("tri", [P, P], BF16, kind="ExternalInput").ap()
    out = nc.dram_tensor("out", [S, D], F32, kind="ExternalOutput").ap()

    with tile.TileContext(nc) as tc:
        with tc.tile_pool(name="const", bufs=1) as cst, \
             tc.tile_pool(name="stage", bufs=3) as stg, \
             tc.tile_pool(name="pt", bufs=3) as ptp, \
             tc.tile_pool(name="small", bufs=2) as sml, \
             tc.tile_pool(name="stgp", bufs=2) as stgp, \
             tc.tile_pool(name="psA", bufs=1, space="PSUM") as psA, \
             tc.tile_pool(name="psC", bufs=1, space="PSUM") as psC:

            mm_ctr = [0]

            def mm_tile(dt=F32):
                i = mm_ctr[0]
                mm_ctr[0] += 1
                return psA.tile([P, 1024], dt, tag=f"s{i % 3}",
                                name=f"mm_s{i % 3}")

            # --- constants (already bf16 from host) ---
            ident_sb = cst.tile([P, P], BF16, tag="ident")
            nc.sync.dma_start(ident_sb[:], ident[:])
            tri_bf = cst.tile([P, P], BF16, tag="tri")
            nc.sync.dma_start(tri_bf[:], tri[:])
            bq_sb = cst.tile([P, OC], F32, tag="bq")
            nc.sync.dma_start(bq_sb[:], bq.rearrange("(c p) -> p c", p=P))
            bk_sb = cst.tile([P, OC], F32, tag="bk")
            nc.sync.dma_start(bk_sb[:], bk.rearrange("(c p) -> p c", p=P))
            bv_sb = cst.tile([HD, NHC], F32, tag="bv")
            nc.sync.dma_start(bv_sb[:], bv.rearrange("(h p) -> p h", p=HD))

            # --- weights: straight bf16 DMA into matmul layouts ---
            w_sbs = {}
            for name, wdram in (("wq", wq), ("wk", wk), ("wv", wv)):
                w_sb = cst.tile([P, FC, DC], BF16, tag=name, name=name)
                w_sbs[name] = w_sb
                nc.gpsimd.dma_start(
                    w_sb[:], wdram.rearrange("(c p) n -> p c n", p=P))
            wo_sb = cst.tile([P, OC, D], BF16, tag="wo")
            nc.gpsimd.dma_start(
                wo_sb[:], wo.rearrange("(c p) n -> p c n", p=P))

            # --- per-window tiles ---
            xT_w, qT_w, kT_w, v_w, ctx_w = [], [], [], [], []
            for j in range(NWIN):
                xT_w.append(cst.tile([P, FC, W], BF16, tag=f"xT{j}",
                                     name=f"xT{j}"))
                qT_w.append(cst.tile([P, OC, W], BF16, tag=f"qT{j}",
                                     name=f"qT{j}"))
                kT_w.append(cst.tile([P, OC, W], BF16, tag=f"kT{j}",
                                     name=f"kT{j}"))
                v_w.append(cst.tile([P, WT, NHC, HD + 1], BF16, tag=f"v{j}",
                                    name=f"v{j}"))
                ctx_w.append(cst.tile([P, NPAIR, W], BF16, tag=f"ctx{j}",
                                      name=f"ctx{j}"))
                nc.vector.memset(v_w[j][:, :, :, HD:HD + 1], 1.0)

            eng_ctr = [0]

            def nxt():
                eng_ctr[0] += 1
                return eng_ctr[0]

            for j in range(NWIN):
                # --- x^T window j: bf16 DMA + bf16 PE transpose (fp32 psum) ---
                for t in range(WT):
                    tokc = j * WT + t
                    st = stg.tile([P, D], BF16, tag="xstage")
                    dma_eng = nc.sync if tokc % 2 == 0 else nc.gpsimd
                    dma_eng.dma_start(st[:], x[tokc * P:(tokc + 1) * P, :])
                    ps = mm_tile(BF16)
                    for fc in range(FC):
                        nc.tensor.transpose(ps[:, fc * P:(fc + 1) * P],
                                            st[:, fc * P:(fc + 1) * P],
                                            ident_sb[:])
                    _copy(nc, nxt(), xT_w[j][:, :, t * P:(t + 1) * P], ps[:])

                # --- Q^T / K^T window j (2 outdim chunks per psum slot) ---
                for dst, wname, b_sb in ((qT_w[j], "wq", bq_sb),
                                         (kT_w[j], "wk", bk_sb)):
                    w_sb = w_sbs[wname]
                    for og in range(OC // 2):
                        ps = mm_tile()
                        for half in range(2):
                            oc = og * 2 + half
                            for fc in range(FC):
                                nc.tensor.matmul(
                                    ps[:, half * W:(half + 1) * W],
                                    w_sb[:, fc, oc * P:(oc + 1) * P],
                                    xT_w[j][:, fc, :],
                                    start=(fc == 0), stop=(fc == FC - 1))
                        for half in range(2):
                            oc = og * 2 + half
                            hv = ps[:, half * W:(half + 1) * W]
                            if nxt() % 2 == 0:
                                nc.vector.tensor_scalar(
                                    dst[:, oc, :], hv,
                                    b_sb[:, oc:oc + 1], None, ALU.add)
                            else:
                                nc.scalar.activation(
                                    dst[:, oc, :], hv, AF.Identity,
                                    bias=b_sb[:, oc:oc + 1])

                # --- V window j (2 token chunks per psum slot) ---
                for tg in range(WT // 2):
                    ps = mm_tile()
                    for half in range(2):
                        t = tg * 2 + half
                        for fc in range(FC):
                            nc.tensor.matmul(
                                ps[:, half * W:(half + 1) * W],
                                xT_w[j][:, fc, t * P:(t + 1) * P],
                                w_sbs["wv"][:, fc, :],
                                start=(fc == 0), stop=(fc == FC - 1))
                    dv = ps.rearrange("p (t h n) -> p t h n", t=2, h=NHC)
                    _copy(nc, nxt(), v_w[j][:, tg * 2:tg * 2 + 2, :, 0:HD], dv)

                # --- attention: all pairs, window j ---
                skc_hi = WT * (j + 1)
                stgw = stgp.tile([HD, NHC, W], BF16, tag="stgw", name="stgw")
                rsw = sml.tile([NHC, W], F32, tag="rsw", name="rsw")
                for p in range(NPAIR):
                    ctx0 = psC.tile([P, W], F32, tag="c0", name="ctx0")
                    ctx1 = psC.tile([P, W], F32, tag="c1", name="ctx1")
                    for skc in range(skc_hi):
                        jk, tk = divmod(skc, WT)
                        rel = skc * P - j * W
                        vs = max(rel, 0)
                        sp = mm_tile()
                        spv = sp.rearrange("p (h n) -> p h n", h=2)
                        nc.tensor.matmul(sp[:, vs:W],
                                         kT_w[jk][0:HD, p, tk * P:(tk + 1) * P],
                                         qT_w[j][0:HD, p, vs:W],
                                         start=True, stop=True)
                        nc.tensor.matmul(sp[:, W + vs:2 * W],
                                         kT_w[jk][HD:P, p, tk * P:(tk + 1) * P],
                                         qT_w[j][HD:P, p, vs:W],
                                         start=True, stop=True)
                        pt = ptp.tile([P, 1024], BF16, tag="pt", name="pt")
                        ptv = pt.rearrange("p (h n) -> p h n", h=2)
                        nc.scalar.activation(ptv[:, :, vs:W], spv[:, :, vs:W],
                                             AF.Exp, scale=SCALE)
                        if rel >= 0:
                            nc.vector.tensor_tensor(
                                ptv[:, :, rel:rel + P], ptv[:, :, rel:rel + P],
                                tri_bf[:, None, :].to_broadcast([P, 2, P]),
                                ALU.mult)
                        st0 = (skc == 0)
                        sp0 = (skc == skc_hi - 1)
                        nc.tensor.matmul(ctx0[0:HD + 1, vs:W],
                                         v_w[jk][:, tk, 2 * p, :],
                                         ptv[:, 0, vs:W], start=st0, stop=sp0)
                        nc.tensor.matmul(ctx1[0:HD + 1, vs:W],
                                         v_w[jk][:, tk, 2 * p + 1, :],
                                         ptv[:, 1, vs:W], start=st0, stop=sp0)

                    # fast drains; rowsums to the window collector
                    for h, ctxp in ((0, ctx0), (1, ctx1)):
                        rw = sml.tile([1, W], F32, tag=f"rw{h}", name="rw")
                        nc.vector.tensor_copy(rw[:], ctxp[HD:HD + 1, :])
                        nc.sync.dma_start(rsw[2 * p + h:2 * p + h + 1, :],
                                          rw[:])
                        _copy(nc, nxt(), stgw[:, 2 * p + h, :], ctxp[0:HD, :])

                # batched reciprocal + normalization for all 8 heads
                rcw = sml.tile([NHC, W], F32, tag="rcw", name="rcw")
                nc.vector.reciprocal(rcw[:], rsw[:])
                for p in range(NPAIR):
                    for h in range(2):
                        i = 2 * p + h
                        bc = sml.tile([HD, W], F32, tag=f"bc{i % 2}",
                                      name="bc")
                        nc.sync.dma_start(
                            bc[:], rcw[i:i + 1, None, :].to_broadcast(
                                [1, HD, W]))
                        if h == 0:
                            dst = ctx_w[j][0:HD, p, :]
                            nc.vector.tensor_tensor(dst, stgw[:, i, :], bc[:],
                                                    ALU.mult)
                            if with_bv:
                                nc.vector.tensor_scalar(
                                    dst, dst, bv_sb[:, i:i + 1], None, ALU.add)
                        else:
                            sh = sml.tile([HD, W], BF16, tag="sh", name="sh")
                            nc.vector.tensor_tensor(sh[:], stgw[:, i, :],
                                                    bc[:], ALU.mult)
                            if with_bv:
                                nc.vector.tensor_scalar(
                                    sh[:], sh[:], bv_sb[:, i:i + 1], None,
                                    ALU.add)
                            nc.sync.dma_start(ctx_w[j][HD:P, p, :], sh[:])

                # --- output projection for window j's tokens ---
                for t in range(WT):
                    tokc = j * WT + t
                    ps = mm_tile()
                    for nb in range(2):
                        for pr in range(NPAIR):
                            nc.tensor.matmul(
                                ps[:, nb * W:(nb + 1) * W],
                                ctx_w[j][:, pr, t * P:(t + 1) * P],
                                wo_sb[:, pr, nb * 512:(nb + 1) * 512],
                                start=(pr == 0), stop=(pr == NPAIR - 1))
                    ost = stg.tile([P, D], F32, tag="ostage")
                    _copy(nc, nxt(), ost[:], ps[:])
                    oeng = nc.sync if t % 2 == 0 else nc.gpsimd
                    oeng.dma_start(out[tokc * P:(tokc + 1) * P, :], ost[:])

    nc.compile()
    return nc


def make_in_maps(x, Wq, bq, Wk, bk, Wv, bv, Wo):
    BF = ml_dtypes.bfloat16
    ident = np.eye(P, dtype=np.float32).astype(BF)
    # tri[p, f] = 1 where f >= p (keep key p for query f within a diag block)
    tri = np.triu(np.ones((P, P), dtype=np.float32)).astype(BF)
    in_maps = []
    for c in range(8):
        b, g = c // 2, c % 2
        sl = slice(g * DC, (g + 1) * DC)
        in_maps.append({
            "x": np.ascontiguousarray(x[b]).astype(BF),
            "wq": np.ascontiguousarray(Wq[:, sl]).astype(BF),
            "wk": np.ascontiguousarray(Wk[:, sl]).astype(BF),
            "wv": np.ascontiguousarray(Wv[:, sl]).astype(BF),
            "wo": np.ascontiguousarray(Wo[sl, :]).astype(BF),
            "bq": np.ascontiguousarray(bq[sl]),
            "bk": np.ascontiguousarray(bk[sl]),
            "bv": np.ascontiguousarray(bv[sl]),
            "ident": ident,
            "tri": tri,
        })
    return in_maps


_NC_CACHE = {}


def kernel(x, Wq, bq, Wk, bk, Wv, bv, Wo, bo):
    x = np.asarray(x, dtype=np.float32)
    args = [np.asarray(a, dtype=np.float32)
            for a in (Wq, bq, Wk, bk, Wv, bv, Wo, bo)]
    Wq, bq, Wk, bk, Wv, bv, Wo, bo = args
    key = ("nc", x.shape[1], bool(np.any(bv)))
    if key not in _NC_CACHE:
        _NC_CACHE[key] = build_nc(S=x.shape[1], num_devices=8,
                                  with_bv=bool(np.any(bv)))
    nc = _NC_CACHE[key]
    in_maps = make_in_maps(x, Wq, bq, Wk, bk, Wv, bv, Wo)
    res = run_bass_kernel_spmd(nc, in_maps, core_ids=list(range(8)))
    B = x.shape[0]
    out = np.empty_like(x)
    for b in range(B):
        out[b] = res.results[2 * b]["out"] + res.results[2 * b + 1]["out"] + bo
    return out


# revision 15
# speedup vs baseline: 1.4796x; 1.3920x over previous
"""Causal multi-head attention layer (train forward) on 8 Trainium2 NeuronCores.

Sharding: batch (4) x head-group (2 of 8 heads each) -> 8 cores.
Per core (batch b, head group g): project Q^T/K^T [512,S] and V [S,512] from
x_b in bf16 (fp32 PSUM accum), run causal attention head-pair-packed on the PE
array (row tiles at partitions 0/64, one [128,1024] PSUM strip pair per key
chunk), softmax rowsums ride a ones column on V (ctx matmul M=65), the
normalization reciprocal is batched per window on DVE and applied during the
ctx drain, then a partial output projection with this core's Wo row block.
Host pre-casts weights/x to bf16, sums the two partials per batch, adds bo.
"""
import numpy as np
import ml_dtypes

import concourse.bass as bass
import concourse.tile as tile
from concourse import bacc, mybir
from concourse.bass_utils import run_bass_kernel_spmd

F32 = mybir.dt.float32
BF16 = mybir.dt.bfloat16
AF = mybir.ActivationFunctionType
ALU = mybir.AluOpType

P = 128
D = 1024          # model dim
DC = 512          # per-core head dims (8 heads x 64)
HD = 64
NHC = 8           # heads per core
NPAIR = 4         # head pairs per core
FC = D // P       # 8 feature chunks
OC = DC // P      # 4 outdim chunks (= head pairs)
W = 512           # query window (fp32 PSUM bank)
WT = W // P       # token chunks per window
SCALE = 1.0 / 32.0  # 1/sqrt(D)


def _copy(nc, i, out, in_):
    if i % 2 == 0:
        nc.vector.tensor_copy(out, in_)
    else:
        nc.scalar.copy(out, in_)


def build_nc(S=2048, num_devices=8, with_bv=False):
    NWIN = S // W

    nc = bacc.Bacc("TRN2", target_bir_lowering=False, debug=False,
                   num_devices=num_devices)
    x = nc.dram_tensor("x", [S, D], BF16, kind="ExternalInput").ap()
    wq = nc.dram_tensor("wq", [D, DC], BF16, kind="ExternalInput").ap()
    wk = nc.dram_tensor("wk", [D, DC], BF16, kind="ExternalInput").ap()
    wv = nc.dram_tensor("wv", [D, DC], BF16, kind="ExternalInput").ap()
    wo = nc.dram_tensor("wo", [DC, D], BF16, kind="ExternalInput").ap()
    bq = nc.dram_tensor("bq", [DC], F32, kind="ExternalInput").ap()
    bk = nc.dram_tensor("bk", [DC], F32, kind="ExternalInput").ap()
    bv = nc.dram_tensor("bv", [DC], F32, kind="ExternalInput").ap()
    ident = nc.dram_tensor("ident", [P, P], BF16, kind="ExternalInput").ap()
    tri = nc.dram_tensor("tri", [P, P], BF16, kind="ExternalInput").ap()
    out = nc.dram_tensor("out", [S, D], F32, kind="ExternalOutput").ap()

    with tile.TileContext(nc) as tc:
        with tc.tile_pool(name="const", bufs=1) as cst, \
             tc.tile_pool(name="stage", bufs=3) as stg, \
             tc.tile_pool(name="pt", bufs=3) as ptp, \
             tc.tile_pool(name="small", bufs=2) as sml, \
             tc.tile_pool(name="stgp", bufs=2) as stgp, \
             tc.tile_pool(name="psA", bufs=1, space="PSUM") as psA, \
             tc.tile_pool(name="psC", bufs=1, space="PSUM") as psC:

            mm_ctr = [0]

            def mm_tile(dt=F32):
                i = mm_ctr[0]
                mm_ctr[0] += 1
                return psA.tile([P, 1024], dt, tag=f"s{i % 3}",
                                name=f"mm_s{i % 3}")

            # --- constants (already bf16 from host) ---
            ident_sb = cst.tile([P, P], BF16, tag="ident")
            nc.sync.dma_start(ident_sb[:], ident[:])
            tri_bf = cst.tile([P, P], BF16, tag="tri")
            nc.sync.dma_start(tri_bf[:], tri[:])
            bq_sb = cst.tile([P, OC], F32, tag="bq")
            nc.sync.dma_start(bq_sb[:], bq.rearrange("(c p) -> p c", p=P))
            bk_sb = cst.tile([P, OC], F32, tag="bk")
            nc.sync.dma_start(bk_sb[:], bk.rearrange("(c p) -> p c", p=P))
            bv_sb = cst.tile([HD, NHC], F32, tag="bv")
            nc.sync.dma_start(bv_sb[:], bv.rearrange("(h p) -> p h", p=HD))

            # --- weights: straight bf16 DMA into matmul layouts ---
            w_sbs = {}
            for name, wdram in (("wq", wq), ("wk", wk), ("wv", wv)):
                w_sb = cst.tile([P, FC, DC], BF16, tag=name, name=name)
                w_sbs[name] = w_sb
                nc.gpsimd.dma_start(
                    w_sb[:], wdram.rearrange("(c p) n -> p c n", p=P))
            wo_sb = cst.tile([P, OC, D], BF16, tag="wo")
            nc.gpsimd.dma_start(
                wo_sb[:], wo.rearrange("(c p) n -> p c n", p=P))

            # --- per-window tiles ---
            xT_w, qT_w, kT_w, v_w, ctx_w = [], [], [], [], []
            for j in range(NWIN):
                xT_w.append(cst.tile([P, FC, W], BF16, tag=f"xT{j}",
                                     name=f"xT{j}"))
                qT_w.append(cst.tile([P, OC, W], BF16, tag=f"qT{j}",
                                     name=f"qT{j}"))
                kT_w.append(cst.tile([P, OC, W], BF16, tag=f"kT{j}",
                                     name=f"kT{j}"))
                v_w.append(cst.tile([P, WT, NHC, HD + 1], BF16, tag=f"v{j}",
                                    name=f"v{j}"))
                ctx_w.append(cst.tile([P, NPAIR, W], BF16, tag=f"ctx{j}",
                                      name=f"ctx{j}"))
                nc.vector.memset(v_w[j][:, :, :, HD:HD + 1], 1.0)

            eng_ctr = [0]

            def nxt():
                eng_ctr[0] += 1
                return eng_ctr[0]

            def emit_proj(j):
                # --- x^T window j: bf16 DMA + bf16 PE transpose (fp32 psum) ---
                for t in range(WT):
                    tokc = j * WT + t
                    st = stg.tile([P, D], BF16, tag="xstage")
                    dma_eng = nc.sync if tokc % 2 == 0 else nc.gpsimd
                    dma_eng.dma_start(st[:], x[tokc * P:(tokc + 1) * P, :])
                    ps = mm_tile(BF16)
                    for fc in range(FC):
                        nc.tensor.transpose(ps[:, fc * P:(fc + 1) * P],
                                            st[:, fc * P:(fc + 1) * P],
                                            ident_sb[:])
                    _copy(nc, nxt(), xT_w[j][:, :, t * P:(t + 1) * P], ps[:])

                # --- Q^T / K^T window j (2 outdim chunks per psum slot) ---
                for dst, wname, b_sb in ((qT_w[j], "wq", bq_sb),
                                         (kT_w[j], "wk", bk_sb)):
                    w_sb = w_sbs[wname]
                    for og in range(OC // 2):
                        ps = mm_tile()
                        for half in range(2):
                            oc = og * 2 + half
                            for fc in range(FC):
                                nc.tensor.matmul(
                                    ps[:, half * W:(half + 1) * W],
                                    w_sb[:, fc, oc * P:(oc + 1) * P],
                                    xT_w[j][:, fc, :],
                                    start=(fc == 0), stop=(fc == FC - 1))
                        for half in range(2):
                            oc = og * 2 + half
                            hv = ps[:, half * W:(half + 1) * W]
                            if nxt() % 2 == 0:
                                nc.vector.tensor_scalar(
                                    dst[:, oc, :], hv,
                                    b_sb[:, oc:oc + 1], None, ALU.add)
                            else:
                                nc.scalar.activation(
                                    dst[:, oc, :], hv, AF.Identity,
                                    bias=b_sb[:, oc:oc + 1])

                # --- V window j (2 token chunks per psum slot) ---
                for tg in range(WT // 2):
                    ps = mm_tile()
                    for half in range(2):
                        t = tg * 2 + half
                        for fc in range(FC):
                            nc.tensor.matmul(
                                ps[:, half * W:(half + 1) * W],
                                xT_w[j][:, fc, t * P:(t + 1) * P],
                                w_sbs["wv"][:, fc, :],
                                start=(fc == 0), stop=(fc == FC - 1))
                    dv = ps.rearrange("p (t h n) -> p t h n", t=2, h=NHC)
                    _copy(nc, nxt(), v_w[j][:, tg * 2:tg * 2 + 2, :, 0:HD], dv)

            def emit_attention(j):
                # --- attention: all pairs, window j ---
                skc_hi = WT * (j + 1)
                stgw = stgp.tile([HD, NHC, W], BF16, tag="stgw", name="stgw")
                rsw = sml.tile([NHC, W], F32, tag="rsw", name="rsw")
                for p in range(NPAIR):
                    ctx0 = psC.tile([P, W], F32, tag="c0", name="ctx0")
                    ctx1 = psC.tile([P, W], F32, tag="c1", name="ctx1")
                    for skc in range(skc_hi):
                        jk, tk = divmod(skc, WT)
                        rel = skc * P - j * W
                        vs = max(rel, 0)
                        sp = mm_tile()
                        spv = sp.rearrange("p (h n) -> p h n", h=2)
                        nc.tensor.matmul(sp[:, vs:W],
                                         kT_w[jk][0:HD, p, tk * P:(tk + 1) * P],
                                         qT_w[j][0:HD, p, vs:W],
                                         start=True, stop=True)
                        nc.tensor.matmul(sp[:, W + vs:2 * W],
                                         kT_w[jk][HD:P, p, tk * P:(tk + 1) * P],
                                         qT_w[j][HD:P, p, vs:W],
                                         start=True, stop=True)
                        pt = ptp.tile([P, 1024], BF16, tag="pt", name="pt")
                        ptv = pt.rearrange("p (h n) -> p h n", h=2)
                        nc.scalar.activation(ptv[:, :, vs:W], spv[:, :, vs:W],
                                             AF.Exp, scale=SCALE)
                        if rel >= 0:
                            nc.vector.tensor_tensor(
                                ptv[:, :, rel:rel + P], ptv[:, :, rel:rel + P],
                                tri_bf[:, None, :].to_broadcast([P, 2, P]),
                                ALU.mult)
                        st0 = (skc == 0)
                        sp0 = (skc == skc_hi - 1)
                        nc.tensor.matmul(ctx0[0:HD + 1, vs:W],
                                         v_w[jk][:, tk, 2 * p, :],
                                         ptv[:, 0, vs:W], start=st0, stop=sp0)
                        nc.tensor.matmul(ctx1[0:HD + 1, vs:W],
                                         v_w[jk][:, tk, 2 * p + 1, :],
                                         ptv[:, 1, vs:W], start=st0, stop=sp0)

                    # fast drains; rowsums to the window collector
                    for h, ctxp in ((0, ctx0), (1, ctx1)):
                        rw = sml.tile([1, W], F32, tag=f"rw{h}", name="rw")
                        nc.vector.tensor_copy(rw[:], ctxp[HD:HD + 1, :])
                        rq = nc.sync if (p + h) % 2 == 0 else nc.gpsimd
                        rq.dma_start(rsw[2 * p + h:2 * p + h + 1, :], rw[:])
                        _copy(nc, nxt(), stgw[:, 2 * p + h, :], ctxp[0:HD, :])

                return stgw, rsw

            def emit_norm(j, stgw, rsw):
                # batched reciprocal + normalization for all 8 heads
                rcf = sml.tile([NHC, W], F32, tag="rcf", name="rcf")
                nc.vector.reciprocal(rcf[:], rsw[:])
                rcw = sml.tile([NHC, W], BF16, tag="rcw", name="rcw")
                nc.vector.tensor_copy(rcw[:], rcf[:])
                for p in range(NPAIR):
                    for h in range(2):
                        i = 2 * p + h
                        bc = sml.tile([HD, W], BF16, tag=f"bc{i % 2}",
                                      name="bc")
                        bq_ = nc.sync if i % 2 == 0 else nc.gpsimd
                        bq_.dma_start(
                            bc[:], rcw[i:i + 1, None, :].to_broadcast(
                                [1, HD, W]))
                        if h == 0:
                            dst = ctx_w[j][0:HD, p, :]
                            nc.vector.tensor_tensor(dst, stgw[:, i, :], bc[:],
                                                    ALU.mult)
                            if with_bv:
                                nc.vector.tensor_scalar(
                                    dst, dst, bv_sb[:, i:i + 1], None, ALU.add)
                        else:
                            sh = sml.tile([HD, W], BF16, tag="sh", name="sh")
                            nc.vector.tensor_tensor(sh[:], stgw[:, i, :],
                                                    bc[:], ALU.mult)
                            if with_bv:
                                nc.vector.tensor_scalar(
                                    sh[:], sh[:], bv_sb[:, i:i + 1], None,
                                    ALU.add)
                            shq = nc.gpsimd if p % 2 == 0 else nc.sync
                            shq.dma_start(ctx_w[j][HD:P, p, :], sh[:])

            def emit_outproj(j):
                # --- output projection for window j's tokens ---
                for t in range(WT):
                    tokc = j * WT + t
                    ps = mm_tile()
                    for nb in range(2):
                        for pr in range(NPAIR):
                            nc.tensor.matmul(
                                ps[:, nb * W:(nb + 1) * W],
                                ctx_w[j][:, pr, t * P:(t + 1) * P],
                                wo_sb[:, pr, nb * 512:(nb + 1) * 512],
                                start=(pr == 0), stop=(pr == NPAIR - 1))
                    ost = stg.tile([P, D], F32, tag="ostage")
                    _copy(nc, nxt(), ost[:], ps[:])
                    oeng = nc.sync if t % 2 == 0 else nc.gpsimd
                    oeng.dma_start(out[tokc * P:(tokc + 1) * P, :], ost[:])

            emit_proj(0)
            for j in range(NWIN):
                stgw, rsw = emit_attention(j)
                if j + 1 < NWIN:
                    emit_proj(j + 1)
                emit_norm(j, stgw, rsw)
                emit_outproj(j)

    nc.compile()
    return nc


def make_in_maps(x, Wq, bq, Wk, bk, Wv, bv, Wo):
    BF = ml_dtypes.bfloat16
    ident = np.eye(P, dtype=np.float32).astype(BF)
    # tri[p, f] = 1 where f >= p (keep key p for query f within a diag block)
    tri = np.triu(np.ones((P, P), dtype=np.float32)).astype(BF)
    in_maps = []
    for c in range(8):
        b, g = c // 2, c % 2
        sl = slice(g * DC, (g + 1) * DC)
        in_maps.append({
            "x": np.ascontiguousarray(x[b]).astype(BF),
            "wq": np.ascontiguousarray(Wq[:, sl]).astype(BF),
            "wk": np.ascontiguousarray(Wk[:, sl]).astype(BF),
            "wv": np.ascontiguousarray(Wv[:, sl]).astype(BF),
            "wo": np.ascontiguousarray(Wo[sl, :]).astype(BF),
            "bq": np.ascontiguousarray(bq[sl]),
            "bk": np.ascontiguousarray(bk[sl]),
            "bv": np.ascontiguousarray(bv[sl]),
            "ident": ident,
            "tri": tri,
        })
    return in_maps


_NC_CACHE = {}


def kernel(x, Wq, bq, Wk, bk, Wv, bv, Wo, bo):
    x = np.asarray(x, dtype=np.float32)
    args = [np.asarray(a, dtype=np.float32)
            for a in (Wq, bq, Wk, bk, Wv, bv, Wo, bo)]
    Wq, bq, Wk, bk, Wv, bv, Wo, bo = args
    key = ("nc", x.shape[1], bool(np.any(bv)))
    if key not in _NC_CACHE:
        _NC_CACHE[key] = build_nc(S=x.shape[1], num_devices=8,
                                  with_bv=bool(np.any(bv)))
    nc = _NC_CACHE[key]
    in_maps = make_in_maps(x, Wq, bq, Wk, bk, Wv, bv, Wo)
    res = run_bass_kernel_spmd(nc, in_maps, core_ids=list(range(8)))
    B = x.shape[0]
    out = np.empty_like(x)
    for b in range(B):
        out[b] = res.results[2 * b]["out"] + res.results[2 * b + 1]["out"] + bo
    return out


# revision 16
# speedup vs baseline: 1.5180x; 1.0259x over previous
"""Causal multi-head attention layer (train forward) on 8 Trainium2 NeuronCores.

Sharding: batch (4) x head-group (2 of 8 heads each) -> 8 cores.
Per core (batch b, head group g): project Q^T/K^T [512,S] and V [S,512] from
x_b in bf16 (fp32 PSUM accum), run causal attention head-pair-packed on the PE
array (row tiles at partitions 0/64, one [128,1024] PSUM strip pair per key
chunk), softmax rowsums ride a ones column on V (ctx matmul M=65), the
normalization reciprocal is batched per window on DVE and applied during the
ctx drain, then a partial output projection with this core's Wo row block.
Host pre-casts weights/x to bf16, sums the two partials per batch, adds bo.
"""
import numpy as np
import ml_dtypes

import concourse.bass as bass
import concourse.tile as tile
from concourse import bacc, mybir
from concourse.bass_utils import run_bass_kernel_spmd

F32 = mybir.dt.float32
BF16 = mybir.dt.bfloat16
AF = mybir.ActivationFunctionType
ALU = mybir.AluOpType

P = 128
D = 1024          # model dim
DC = 512          # per-core head dims (8 heads x 64)
HD = 64
NHC = 8           # heads per core
NPAIR = 4         # head pairs per core
FC = D // P       # 8 feature chunks
OC = DC // P      # 4 outdim chunks (= head pairs)
W = 512           # query window (fp32 PSUM bank)
WT = W // P       # token chunks per window
SCALE = 1.0 / 32.0  # 1/sqrt(D)


def _copy(nc, i, out, in_):
    if i % 2 == 0:
        nc.vector.tensor_copy(out, in_)
    else:
        nc.scalar.copy(out, in_)


def build_nc(S=2048, num_devices=8, with_bv=False):
    NWIN = S // W

    nc = bacc.Bacc("TRN2", target_bir_lowering=False, debug=False,
                   num_devices=num_devices)
    x = nc.dram_tensor("x", [S, D], BF16, kind="ExternalInput").ap()
    wq = nc.dram_tensor("wq", [D, DC], BF16, kind="ExternalInput").ap()
    wk = nc.dram_tensor("wk", [D, DC], BF16, kind="ExternalInput").ap()
    wv = nc.dram_tensor("wv", [D, DC], BF16, kind="ExternalInput").ap()
    wo = nc.dram_tensor("wo", [DC, D], BF16, kind="ExternalInput").ap()
    bq = nc.dram_tensor("bq", [DC], F32, kind="ExternalInput").ap()
    bk = nc.dram_tensor("bk", [DC], F32, kind="ExternalInput").ap()
    bv = nc.dram_tensor("bv", [DC], F32, kind="ExternalInput").ap()
    tri = nc.dram_tensor("tri", [P, P], BF16, kind="ExternalInput").ap()
    out = nc.dram_tensor("out", [S, D], F32, kind="ExternalOutput").ap()

    with tile.TileContext(nc) as tc:
        with tc.tile_pool(name="const", bufs=1) as cst, \
             tc.tile_pool(name="stage", bufs=3) as stg, \
             tc.tile_pool(name="pt", bufs=3) as ptp, \
             tc.tile_pool(name="small", bufs=2) as sml, \
             tc.tile_pool(name="stgp", bufs=2) as stgp, \
             tc.tile_pool(name="psA", bufs=1, space="PSUM") as psA, \
             tc.tile_pool(name="psC", bufs=1, space="PSUM") as psC:

            mm_ctr = [0]

            def mm_tile(dt=F32):
                i = mm_ctr[0]
                mm_ctr[0] += 1
                return psA.tile([P, 1024], dt, tag=f"s{i % 3}",
                                name=f"mm_s{i % 3}")

            # --- constants (already bf16 from host) ---
            tri_bf = cst.tile([P, P], BF16, tag="tri")
            nc.sync.dma_start(tri_bf[:], tri[:])
            bq_sb = cst.tile([P, OC], F32, tag="bq")
            nc.sync.dma_start(bq_sb[:], bq.rearrange("(c p) -> p c", p=P))
            bk_sb = cst.tile([P, OC], F32, tag="bk")
            nc.sync.dma_start(bk_sb[:], bk.rearrange("(c p) -> p c", p=P))
            bv_sb = cst.tile([HD, NHC], F32, tag="bv")
            nc.sync.dma_start(bv_sb[:], bv.rearrange("(h p) -> p h", p=HD))

            # --- weights: straight bf16 DMA into matmul layouts ---
            w_sbs = {}
            for name, wdram in (("wq", wq), ("wk", wk), ("wv", wv)):
                w_sb = cst.tile([P, FC, DC], BF16, tag=name, name=name)
                w_sbs[name] = w_sb
                nc.gpsimd.dma_start(
                    w_sb[:], wdram.rearrange("(c p) n -> p c n", p=P))
            wo_sb = cst.tile([P, OC, D], BF16, tag="wo")
            nc.gpsimd.dma_start(
                wo_sb[:], wo.rearrange("(c p) n -> p c n", p=P))

            # --- per-window tiles ---
            xT_w, qT_w, kT_w, v_w, ctx_w = [], [], [], [], []
            for j in range(NWIN):
                xT_w.append(cst.tile([P, FC, W], BF16, tag=f"xT{j}",
                                     name=f"xT{j}"))
                qT_w.append(cst.tile([P, OC, W], BF16, tag=f"qT{j}",
                                     name=f"qT{j}"))
                kT_w.append(cst.tile([P, OC, W], BF16, tag=f"kT{j}",
                                     name=f"kT{j}"))
                v_w.append(cst.tile([P, WT, NHC, HD + 1], BF16, tag=f"v{j}",
                                    name=f"v{j}"))
                ctx_w.append(cst.tile([P, NPAIR, W], BF16, tag=f"ctx{j}",
                                      name=f"ctx{j}"))
                nc.vector.memset(v_w[j][:, :, :, HD:HD + 1], 1.0)

            eng_ctr = [0]

            def nxt():
                eng_ctr[0] += 1
                return eng_ctr[0]

            def emit_proj(j):
                # --- x^T window j: XBAR DMA transpose straight from DRAM ---
                for t in range(WT):
                    tokc = j * WT + t
                    nc.sync.dma_start_transpose(
                        xT_w[j][:, :, t * P:(t + 1) * P],
                        x[tokc * P:(tokc + 1) * P, :])

                # --- Q^T / K^T window j (2 outdim chunks per psum slot) ---
                for dst, wname, b_sb in ((qT_w[j], "wq", bq_sb),
                                         (kT_w[j], "wk", bk_sb)):
                    w_sb = w_sbs[wname]
                    for og in range(OC // 2):
                        ps = mm_tile()
                        for half in range(2):
                            oc = og * 2 + half
                            for fc in range(FC):
                                nc.tensor.matmul(
                                    ps[:, half * W:(half + 1) * W],
                                    w_sb[:, fc, oc * P:(oc + 1) * P],
                                    xT_w[j][:, fc, :],
                                    start=(fc == 0), stop=(fc == FC - 1))
                        for half in range(2):
                            oc = og * 2 + half
                            hv = ps[:, half * W:(half + 1) * W]
                            if nxt() % 2 == 0:
                                nc.vector.tensor_scalar(
                                    dst[:, oc, :], hv,
                                    b_sb[:, oc:oc + 1], None, ALU.add)
                            else:
                                nc.scalar.activation(
                                    dst[:, oc, :], hv, AF.Identity,
                                    bias=b_sb[:, oc:oc + 1])

                # --- V window j (2 token chunks per psum slot) ---
                for tg in range(WT // 2):
                    ps = mm_tile()
                    for half in range(2):
                        t = tg * 2 + half
                        for fc in range(FC):
                            nc.tensor.matmul(
                                ps[:, half * W:(half + 1) * W],
                                xT_w[j][:, fc, t * P:(t + 1) * P],
                                w_sbs["wv"][:, fc, :],
                                start=(fc == 0), stop=(fc == FC - 1))
                    dv = ps.rearrange("p (t h n) -> p t h n", t=2, h=NHC)
                    _copy(nc, nxt(), v_w[j][:, tg * 2:tg * 2 + 2, :, 0:HD], dv)

            def emit_attention(j, pairs, stgw=None, rsw=None):
                # --- attention for the given head pairs, window j ---
                skc_hi = WT * (j + 1)
                if stgw is None:
                    stgw = stgp.tile([HD, NHC, W], BF16, tag="stgw",
                                     name="stgw")
                    rsw = sml.tile([NHC, W], F32, tag="rsw", name="rsw")
                for p in pairs:
                    ctx0 = psC.tile([P, W], F32, tag="c0", name="ctx0")
                    ctx1 = psC.tile([P, W], F32, tag="c1", name="ctx1")
                    for skc in range(skc_hi):
                        jk, tk = divmod(skc, WT)
                        rel = skc * P - j * W
                        vs = max(rel, 0)
                        sp = mm_tile()
                        spv = sp.rearrange("p (h n) -> p h n", h=2)
                        nc.tensor.matmul(sp[:, vs:W],
                                         kT_w[jk][0:HD, p, tk * P:(tk + 1) * P],
                                         qT_w[j][0:HD, p, vs:W],
                                         start=True, stop=True)
                        nc.tensor.matmul(sp[:, W + vs:2 * W],
                                         kT_w[jk][HD:P, p, tk * P:(tk + 1) * P],
                                         qT_w[j][HD:P, p, vs:W],
                                         start=True, stop=True)
                        pt = ptp.tile([P, 1024], BF16, tag="pt", name="pt")
                        ptv = pt.rearrange("p (h n) -> p h n", h=2)
                        nc.scalar.activation(ptv[:, :, vs:W], spv[:, :, vs:W],
                                             AF.Exp, scale=SCALE)
                        if rel >= 0:
                            nc.vector.tensor_tensor(
                                ptv[:, :, rel:rel + P], ptv[:, :, rel:rel + P],
                                tri_bf[:, None, :].to_broadcast([P, 2, P]),
                                ALU.mult)
                        st0 = (skc == 0)
                        sp0 = (skc == skc_hi - 1)
                        nc.tensor.matmul(ctx0[0:HD + 1, vs:W],
                                         v_w[jk][:, tk, 2 * p, :],
                                         ptv[:, 0, vs:W], start=st0, stop=sp0)
                        nc.tensor.matmul(ctx1[0:HD + 1, vs:W],
                                         v_w[jk][:, tk, 2 * p + 1, :],
                                         ptv[:, 1, vs:W], start=st0, stop=sp0)

                    # fast drains; rowsums to the window collector
                    for h, ctxp in ((0, ctx0), (1, ctx1)):
                        rw = sml.tile([1, W], F32, tag=f"rw{h}", name="rw")
                        nc.vector.tensor_copy(rw[:], ctxp[HD:HD + 1, :])
                        ri = 2 * (p - pairs[0]) + h
                        rq = nc.sync if (p + h) % 2 == 0 else nc.gpsimd
                        rq.dma_start(rsw[ri:ri + 1, :], rw[:])
                        _copy(nc, nxt(), stgw[:, 2 * p + h, :], ctxp[0:HD, :])

                return stgw, rsw

            def emit_norm(j, stgw, rsw, pairs):
                # batched reciprocal + normalization for the given pairs
                nr = 2 * len(pairs)
                rcf = sml.tile([NHC, W], F32, tag="rcf", name="rcf")
                nc.vector.reciprocal(rcf[0:nr, :], rsw[0:nr, :])
                rcw = sml.tile([NHC, W], BF16, tag="rcw", name="rcw")
                nc.vector.tensor_copy(rcw[0:nr, :], rcf[0:nr, :])
                for p in pairs:
                    for h in range(2):
                        i = 2 * p + h
                        ri = 2 * (p - pairs[0]) + h
                        bc = sml.tile([HD, W], BF16, tag=f"bc{i % 2}",
                                      name="bc")
                        bq_ = nc.sync if i % 2 == 0 else nc.gpsimd
                        bq_.dma_start(
                            bc[:], rcw[ri:ri + 1, None, :].to_broadcast(
                                [1, HD, W]))
                        if h == 0:
                            dst = ctx_w[j][0:HD, p, :]
                            nc.vector.tensor_tensor(dst, stgw[:, i, :], bc[:],
                                                    ALU.mult)
                            if with_bv:
                                nc.vector.tensor_scalar(
                                    dst, dst, bv_sb[:, i:i + 1], None, ALU.add)
                        else:
                            sh = sml.tile([HD, W], BF16, tag="sh", name="sh")
                            nc.vector.tensor_tensor(sh[:], stgw[:, i, :],
                                                    bc[:], ALU.mult)
                            if with_bv:
                                nc.vector.tensor_scalar(
                                    sh[:], sh[:], bv_sb[:, i:i + 1], None,
                                    ALU.add)
                            shq = nc.gpsimd if p % 2 == 0 else nc.sync
                            shq.dma_start(ctx_w[j][HD:P, p, :], sh[:])

            def emit_outproj(j):
                # --- output projection for window j's tokens ---
                for t in range(WT):
                    tokc = j * WT + t
                    ps = mm_tile()
                    for nb in range(2):
                        for pr in range(NPAIR):
                            nc.tensor.matmul(
                                ps[:, nb * W:(nb + 1) * W],
                                ctx_w[j][:, pr, t * P:(t + 1) * P],
                                wo_sb[:, pr, nb * 512:(nb + 1) * 512],
                                start=(pr == 0), stop=(pr == NPAIR - 1))
                    ost = stg.tile([P, D], F32, tag="ostage")
                    _copy(nc, nxt(), ost[:], ps[:])
                    oeng = nc.sync if t % 2 == 0 else nc.gpsimd
                    oeng.dma_start(out[tokc * P:(tokc + 1) * P, :], ost[:])

            emit_proj(0)
            for j in range(NWIN):
                if j + 1 < NWIN:
                    stgw, rsw = emit_attention(j, list(range(NPAIR)))
                    emit_proj(j + 1)
                    emit_norm(j, stgw, rsw, list(range(NPAIR)))
                else:
                    half = NPAIR // 2
                    stgw, rsw = emit_attention(j, list(range(half)))
                    emit_norm(j, stgw, rsw, list(range(half)))
                    stgw2, rsw2 = emit_attention(j, list(range(half, NPAIR)))
                    emit_norm(j, stgw2, rsw2, list(range(half, NPAIR)))
                emit_outproj(j)

    nc.compile()
    return nc


def make_in_maps(x, Wq, bq, Wk, bk, Wv, bv, Wo):
    BF = ml_dtypes.bfloat16
    # tri[p, f] = 1 where f >= p (keep key p for query f within a diag block)
    tri = np.triu(np.ones((P, P), dtype=np.float32)).astype(BF)
    in_maps = []
    for c in range(8):
        b, g = c // 2, c % 2
        sl = slice(g * DC, (g + 1) * DC)
        in_maps.append({
            "x": np.ascontiguousarray(x[b]).astype(BF),
            "wq": np.ascontiguousarray(Wq[:, sl]).astype(BF),
            "wk": np.ascontiguousarray(Wk[:, sl]).astype(BF),
            "wv": np.ascontiguousarray(Wv[:, sl]).astype(BF),
            "wo": np.ascontiguousarray(Wo[sl, :]).astype(BF),
            "bq": np.ascontiguousarray(bq[sl]),
            "bk": np.ascontiguousarray(bk[sl]),
            "bv": np.ascontiguousarray(bv[sl]),
            "tri": tri,
        })
    return in_maps


_NC_CACHE = {}


def kernel(x, Wq, bq, Wk, bk, Wv, bv, Wo, bo):
    x = np.asarray(x, dtype=np.float32)
    args = [np.asarray(a, dtype=np.float32)
            for a in (Wq, bq, Wk, bk, Wv, bv, Wo, bo)]
    Wq, bq, Wk, bk, Wv, bv, Wo, bo = args
    key = ("nc", x.shape[1], bool(np.any(bv)))
    if key not in _NC_CACHE:
        _NC_CACHE[key] = build_nc(S=x.shape[1], num_devices=8,
                                  with_bv=bool(np.any(bv)))
    nc = _NC_CACHE[key]
    in_maps = make_in_maps(x, Wq, bq, Wk, bk, Wv, bv, Wo)
    res = run_bass_kernel_spmd(nc, in_maps, core_ids=list(range(8)))
    B = x.shape[0]
    out = np.empty_like(x)
    for b in range(B):
        out[b] = res.results[2 * b]["out"] + res.results[2 * b + 1]["out"] + bo
    return out


# revision 17
# speedup vs baseline: 1.5343x; 1.0107x over previous
"""Causal multi-head attention layer (train forward) on 8 Trainium2 NeuronCores.

Sharding: batch (4) x head-group (2 of 8 heads each) -> 8 cores.
Per core (batch b, head group g): project Q^T/K^T [512,S] and V [S,512] from
x_b in bf16 (fp32 PSUM accum), run causal attention head-pair-packed on the PE
array (row tiles at partitions 0/64, one [128,1024] PSUM strip pair per key
chunk), softmax rowsums ride a ones column on V (ctx matmul M=65), the
normalization reciprocal is batched per window on DVE and applied during the
ctx drain, then a partial output projection with this core's Wo row block.
Host pre-casts weights/x to bf16, sums the two partials per batch, adds bo.
"""
import numpy as np
import ml_dtypes

import concourse.bass as bass
import concourse.tile as tile
from concourse import bacc, mybir
from concourse.bass_utils import run_bass_kernel_spmd

F32 = mybir.dt.float32
BF16 = mybir.dt.bfloat16
AF = mybir.ActivationFunctionType
ALU = mybir.AluOpType

P = 128
D = 1024          # model dim
DC = 512          # per-core head dims (8 heads x 64)
HD = 64
NHC = 8           # heads per core
NPAIR = 4         # head pairs per core
FC = D // P       # 8 feature chunks
OC = DC // P      # 4 outdim chunks (= head pairs)
W = 512           # query window (fp32 PSUM bank)
WT = W // P       # token chunks per window
SCALE = 1.0 / 32.0  # 1/sqrt(D)


def _copy(nc, i, out, in_):
    if i % 2 == 0:
        nc.vector.tensor_copy(out, in_)
    else:
        nc.scalar.copy(out, in_)


def build_nc(S=2048, num_devices=8, with_bv=False):
    NWIN = S // W

    nc = bacc.Bacc("TRN2", target_bir_lowering=False, debug=False,
                   num_devices=num_devices)
    x = nc.dram_tensor("x", [S, D], BF16, kind="ExternalInput").ap()
    wq = nc.dram_tensor("wq", [D, DC], BF16, kind="ExternalInput").ap()
    wk = nc.dram_tensor("wk", [D, DC], BF16, kind="ExternalInput").ap()
    wv = nc.dram_tensor("wv", [D, DC], BF16, kind="ExternalInput").ap()
    wo = nc.dram_tensor("wo", [DC, D], BF16, kind="ExternalInput").ap()
    bq = nc.dram_tensor("bq", [DC], F32, kind="ExternalInput").ap()
    bk = nc.dram_tensor("bk", [DC], F32, kind="ExternalInput").ap()
    bv = nc.dram_tensor("bv", [DC], F32, kind="ExternalInput").ap()
    tri = nc.dram_tensor("tri", [P, P], BF16, kind="ExternalInput").ap()
    out = nc.dram_tensor("out", [S, D], F32, kind="ExternalOutput").ap()

    with tile.TileContext(nc) as tc:
        with tc.tile_pool(name="const", bufs=1) as cst, \
             tc.tile_pool(name="stage", bufs=3) as stg, \
             tc.tile_pool(name="pt", bufs=3) as ptp, \
             tc.tile_pool(name="small", bufs=2) as sml, \
             tc.tile_pool(name="stgp", bufs=2) as stgp, \
             tc.tile_pool(name="psA", bufs=1, space="PSUM") as psA, \
             tc.tile_pool(name="psC", bufs=1, space="PSUM") as psC:

            mm_ctr = [0]

            def mm_tile(dt=F32):
                i = mm_ctr[0]
                mm_ctr[0] += 1
                return psA.tile([P, 1024], dt, tag=f"s{i % 3}",
                                name=f"mm_s{i % 3}")

            # --- constants (already bf16 from host) ---
            tri_bf = cst.tile([P, P], BF16, tag="tri")
            nc.sync.dma_start(tri_bf[:], tri[:])
            bq_sb = cst.tile([P, OC], F32, tag="bq")
            nc.sync.dma_start(bq_sb[:], bq.rearrange("(c p) -> p c", p=P))
            bk_sb = cst.tile([P, OC], F32, tag="bk")
            nc.sync.dma_start(bk_sb[:], bk.rearrange("(c p) -> p c", p=P))
            bv_sb = cst.tile([HD, NHC], F32, tag="bv")
            nc.sync.dma_start(bv_sb[:], bv.rearrange("(h p) -> p h", p=HD))

            # --- weights: straight bf16 DMA into matmul layouts ---
            w_sbs = {}
            for wi, (name, wdram) in enumerate(
                    (("wq", wq), ("wk", wk), ("wv", wv))):
                w_sb = cst.tile([P, FC, DC], BF16, tag=name, name=name)
                w_sbs[name] = w_sb
                for fc in range(FC):
                    wq_ = nc.sync if (wi + fc) % 2 == 0 else nc.gpsimd
                    wq_.dma_start(w_sb[:, fc, :],
                                  wdram[fc * P:(fc + 1) * P, :])
            wo_sb = cst.tile([P, OC, D], BF16, tag="wo")
            for c in range(OC):
                wq_ = nc.sync if c % 2 == 0 else nc.gpsimd
                wq_.dma_start(wo_sb[:, c, :], wo[c * P:(c + 1) * P, :])

            # --- per-window tiles ---
            xT_w, qT_w, kT_w, v_w, ctx_w = [], [], [], [], []
            for j in range(NWIN):
                xT_w.append(cst.tile([P, FC, W], BF16, tag=f"xT{j}",
                                     name=f"xT{j}"))
                qT_w.append(cst.tile([P, OC, W], BF16, tag=f"qT{j}",
                                     name=f"qT{j}"))
                kT_w.append(cst.tile([P, OC, W], BF16, tag=f"kT{j}",
                                     name=f"kT{j}"))
                v_w.append(cst.tile([P, WT, NHC, HD + 1], BF16, tag=f"v{j}",
                                    name=f"v{j}"))
                ctx_w.append(cst.tile([P, NPAIR, W], BF16, tag=f"ctx{j}",
                                      name=f"ctx{j}"))
                nc.vector.memset(v_w[j][:, :, :, HD:HD + 1], 1.0)

            eng_ctr = [0]

            def nxt():
                eng_ctr[0] += 1
                return eng_ctr[0]

            def emit_proj(j):
                # --- x^T window j: XBAR DMA transpose straight from DRAM ---
                for t in range(WT):
                    tokc = j * WT + t
                    nc.sync.dma_start_transpose(
                        xT_w[j][:, :, t * P:(t + 1) * P],
                        x[tokc * P:(tokc + 1) * P, :])

                # --- Q^T / K^T window j (2 outdim chunks per psum slot) ---
                for dst, wname, b_sb in ((qT_w[j], "wq", bq_sb),
                                         (kT_w[j], "wk", bk_sb)):
                    w_sb = w_sbs[wname]
                    for og in range(OC // 2):
                        ps = mm_tile()
                        for half in range(2):
                            oc = og * 2 + half
                            for fc in range(FC):
                                nc.tensor.matmul(
                                    ps[:, half * W:(half + 1) * W],
                                    w_sb[:, fc, oc * P:(oc + 1) * P],
                                    xT_w[j][:, fc, :],
                                    start=(fc == 0), stop=(fc == FC - 1))
                        for half in range(2):
                            oc = og * 2 + half
                            hv = ps[:, half * W:(half + 1) * W]
                            if nxt() % 2 == 0:
                                nc.vector.tensor_scalar(
                                    dst[:, oc, :], hv,
                                    b_sb[:, oc:oc + 1], None, ALU.add)
                            else:
                                nc.scalar.activation(
                                    dst[:, oc, :], hv, AF.Identity,
                                    bias=b_sb[:, oc:oc + 1])

                # --- V window j (2 token chunks per psum slot) ---
                for tg in range(WT // 2):
                    ps = mm_tile()
                    for half in range(2):
                        t = tg * 2 + half
                        for fc in range(FC):
                            nc.tensor.matmul(
                                ps[:, half * W:(half + 1) * W],
                                xT_w[j][:, fc, t * P:(t + 1) * P],
                                w_sbs["wv"][:, fc, :],
                                start=(fc == 0), stop=(fc == FC - 1))
                    dv = ps.rearrange("p (t h n) -> p t h n", t=2, h=NHC)
                    _copy(nc, nxt(), v_w[j][:, tg * 2:tg * 2 + 2, :, 0:HD], dv)

            def emit_attention(j, pairs, stgw=None, rsw=None):
                # --- attention for the given head pairs, window j ---
                skc_hi = WT * (j + 1)
                if stgw is None:
                    stgw = stgp.tile([HD, NHC, W], BF16, tag="stgw",
                                     name="stgw")
                    rsw = sml.tile([NHC, W], F32, tag="rsw", name="rsw")
                for p in pairs:
                    ctx0 = psC.tile([P, W], F32, tag="c0", name="ctx0")
                    ctx1 = psC.tile([P, W], F32, tag="c1", name="ctx1")
                    for skc in range(skc_hi):
                        jk, tk = divmod(skc, WT)
                        rel = skc * P - j * W
                        vs = max(rel, 0)
                        sp = mm_tile()
                        spv = sp.rearrange("p (h n) -> p h n", h=2)
                        nc.tensor.matmul(sp[:, vs:W],
                                         kT_w[jk][0:HD, p, tk * P:(tk + 1) * P],
                                         qT_w[j][0:HD, p, vs:W],
                                         start=True, stop=True)
                        nc.tensor.matmul(sp[:, W + vs:2 * W],
                                         kT_w[jk][HD:P, p, tk * P:(tk + 1) * P],
                                         qT_w[j][HD:P, p, vs:W],
                                         start=True, stop=True)
                        pt = ptp.tile([P, 1024], BF16, tag="pt", name="pt")
                        ptv = pt.rearrange("p (h n) -> p h n", h=2)
                        nc.scalar.activation(ptv[:, :, vs:W], spv[:, :, vs:W],
                                             AF.Exp, scale=SCALE)
                        if rel >= 0:
                            nc.vector.tensor_tensor(
                                ptv[:, :, rel:rel + P], ptv[:, :, rel:rel + P],
                                tri_bf[:, None, :].to_broadcast([P, 2, P]),
                                ALU.mult)
                        st0 = (skc == 0)
                        sp0 = (skc == skc_hi - 1)
                        nc.tensor.matmul(ctx0[0:HD + 1, vs:W],
                                         v_w[jk][:, tk, 2 * p, :],
                                         ptv[:, 0, vs:W], start=st0, stop=sp0)
                        nc.tensor.matmul(ctx1[0:HD + 1, vs:W],
                                         v_w[jk][:, tk, 2 * p + 1, :],
                                         ptv[:, 1, vs:W], start=st0, stop=sp0)

                    # fast drains; rowsums to the window collector
                    for h, ctxp in ((0, ctx0), (1, ctx1)):
                        rw = sml.tile([1, W], F32, tag=f"rw{h}", name="rw")
                        nc.vector.tensor_copy(rw[:], ctxp[HD:HD + 1, :])
                        ri = 2 * (p - pairs[0]) + h
                        rq = nc.sync if (p + h) % 2 == 0 else nc.gpsimd
                        rq.dma_start(rsw[ri:ri + 1, :], rw[:])
                        _copy(nc, nxt(), stgw[:, 2 * p + h, :], ctxp[0:HD, :])

                return stgw, rsw

            def emit_norm(j, stgw, rsw, pairs):
                # batched reciprocal + normalization for the given pairs
                nr = 2 * len(pairs)
                rcf = sml.tile([NHC, W], F32, tag="rcf", name="rcf")
                nc.vector.reciprocal(rcf[0:nr, :], rsw[0:nr, :])
                rcw = sml.tile([NHC, W], BF16, tag="rcw", name="rcw")
                nc.vector.tensor_copy(rcw[0:nr, :], rcf[0:nr, :])
                for p in pairs:
                    for h in range(2):
                        i = 2 * p + h
                        ri = 2 * (p - pairs[0]) + h
                        bc = sml.tile([HD, W], BF16, tag=f"bc{i % 2}",
                                      name="bc")
                        bq_ = nc.sync if i % 2 == 0 else nc.gpsimd
                        bq_.dma_start(
                            bc[:], rcw[ri:ri + 1, None, :].to_broadcast(
                                [1, HD, W]))
                        if h == 0:
                            dst = ctx_w[j][0:HD, p, :]
                            nc.vector.tensor_tensor(dst, stgw[:, i, :], bc[:],
                                                    ALU.mult)
                            if with_bv:
                                nc.vector.tensor_scalar(
                                    dst, dst, bv_sb[:, i:i + 1], None, ALU.add)
                        else:
                            sh = sml.tile([HD, W], BF16, tag="sh", name="sh")
                            nc.vector.tensor_tensor(sh[:], stgw[:, i, :],
                                                    bc[:], ALU.mult)
                            if with_bv:
                                nc.vector.tensor_scalar(
                                    sh[:], sh[:], bv_sb[:, i:i + 1], None,
                                    ALU.add)
                            shq = nc.gpsimd if p % 2 == 0 else nc.sync
                            shq.dma_start(ctx_w[j][HD:P, p, :], sh[:])

            def emit_outproj(j):
                # --- output projection for window j's tokens ---
                for t in range(WT):
                    tokc = j * WT + t
                    ps = mm_tile()
                    for nb in range(2):
                        for pr in range(NPAIR):
                            nc.tensor.matmul(
                                ps[:, nb * W:(nb + 1) * W],
                                ctx_w[j][:, pr, t * P:(t + 1) * P],
                                wo_sb[:, pr, nb * 512:(nb + 1) * 512],
                                start=(pr == 0), stop=(pr == NPAIR - 1))
                    ost = stg.tile([P, D], F32, tag="ostage")
                    _copy(nc, nxt(), ost[:], ps[:])
                    oeng = nc.sync if t % 2 == 0 else nc.gpsimd
                    oeng.dma_start(out[tokc * P:(tokc + 1) * P, :], ost[:])

            emit_proj(0)
            for j in range(NWIN):
                if j + 1 < NWIN:
                    stgw, rsw = emit_attention(j, list(range(NPAIR)))
                    emit_proj(j + 1)
                    emit_norm(j, stgw, rsw, list(range(NPAIR)))
                else:
                    half = NPAIR // 2
                    stgw, rsw = emit_attention(j, list(range(half)))
                    stgw2, rsw2 = emit_attention(j, list(range(half, NPAIR)))
                    emit_norm(j, stgw, rsw, list(range(half)))
                    emit_norm(j, stgw2, rsw2, list(range(half, NPAIR)))
                emit_outproj(j)

    nc.compile()
    return nc


def make_in_maps(x, Wq, bq, Wk, bk, Wv, bv, Wo):
    BF = ml_dtypes.bfloat16
    # tri[p, f] = 1 where f >= p (keep key p for query f within a diag block)
    tri = np.triu(np.ones((P, P), dtype=np.float32)).astype(BF)
    in_maps = []
    for c in range(8):
        b, g = c // 2, c % 2
        sl = slice(g * DC, (g + 1) * DC)
        in_maps.append({
            "x": np.ascontiguousarray(x[b]).astype(BF),
            "wq": np.ascontiguousarray(Wq[:, sl]).astype(BF),
            "wk": np.ascontiguousarray(Wk[:, sl]).astype(BF),
            "wv": np.ascontiguousarray(Wv[:, sl]).astype(BF),
            "wo": np.ascontiguousarray(Wo[sl, :]).astype(BF),
            "bq": np.ascontiguousarray(bq[sl]),
            "bk": np.ascontiguousarray(bk[sl]),
            "bv": np.ascontiguousarray(bv[sl]),
            "tri": tri,
        })
    return in_maps


_NC_CACHE = {}


def kernel(x, Wq, bq, Wk, bk, Wv, bv, Wo, bo):
    x = np.asarray(x, dtype=np.float32)
    args = [np.asarray(a, dtype=np.float32)
            for a in (Wq, bq, Wk, bk, Wv, bv, Wo, bo)]
    Wq, bq, Wk, bk, Wv, bv, Wo, bo = args
    key = ("nc", x.shape[1], bool(np.any(bv)))
    if key not in _NC_CACHE:
        _NC_CACHE[key] = build_nc(S=x.shape[1], num_devices=8,
                                  with_bv=bool(np.any(bv)))
    nc = _NC_CACHE[key]
    in_maps = make_in_maps(x, Wq, bq, Wk, bk, Wv, bv, Wo)
    res = run_bass_kernel_spmd(nc, in_maps, core_ids=list(range(8)))
    B = x.shape[0]
    out = np.empty_like(x)
    for b in range(B):
        out[b] = res.results[2 * b]["out"] + res.results[2 * b + 1]["out"] + bo
    return out
